# revision 1
# baseline (speedup 1.0000x reference)
"""3-layer GCN node classifier on 8 Trainium2 NeuronCores.

Math (per layer, folding the symmetric normalization):
    deg[v]  = in-degree(v) (with self loop), dinv = rsqrt(deg)
    g       = dinv * (h @ W)                  (rows scaled)
    out[c]  = dinv[c] * ( sum_{e: col=c} g[row_e] + g[c] ) + b
    h_next  = relu(out)      (layers 1,2; layer 3 has no relu)

Distribution: nodes are range-sharded across 8 cores (graph parallel).
Each core computes g for its own nodes (dense matmul), the g-table is
all-gathered to every core's HBM, each core then gathers the rows for
the edges whose *destination* it owns (SWDGE dma_gather) and
scatter-adds them into per-destination-tile PSUM accumulators via
one-hot matmuls on the TensorEngine.

Host-side preprocessing only touches edge_index (graph structure):
CSR-style bucketing of edges by destination tile, degree computation,
a within-core node permutation that load-balances destination tiles,
and int16 gather-index packing (the 50k-row table is split into two
overlapping <=32768-row views because SWDGE gather indices are int16).
"""

import math
import os
import numpy as np

# ---------------------------------------------------------------------------
# problem constants (hardcoded per contract; kernel.py must be self-contained)
# ---------------------------------------------------------------------------
N_NODES = 50000
IN_C, HID_C, OUT_C = 128, 128, 64
M_CORES = 8
NPC = N_NODES // M_CORES            # 6250 nodes per core
TPC = (NPC + 127) // 128            # 49 destination tiles per core
NPAD = TPC * 128                    # 6272 padded nodes per core
TBL = M_CORES * NPAD                # 50176 rows in the all-gathered table
LO_CAP = 32768                      # int16 index reach
HI_OFF = max(0, TBL - 32768)        # 17408: hi view = table[HI_OFF:]

F32 = "float32"


# ---------------------------------------------------------------------------
# host-side graph preprocessing (indices only)
# ---------------------------------------------------------------------------
def _plan(edge_index: np.ndarray):
    """Build per-core index/metadata arrays from edge_index [2, E]."""
    row = np.asarray(edge_index[0], dtype=np.int64)
    col = np.asarray(edge_index[1], dtype=np.int64)

    deg_in = np.bincount(col, minlength=N_NODES)          # edges only
    dinv = 1.0 / np.sqrt(deg_in + 1.0)                     # + self loop

    # within-core permutation: snake-fill tiles with degree-sorted nodes so
    # every destination tile carries a near-equal number of incoming edges.
    pos_local = np.empty(N_NODES, dtype=np.int64)
    for k in range(M_CORES):
        v0 = k * NPC
        d = deg_in[v0 : v0 + NPC]
        order = np.argsort(-d, kind="stable")              # heavy first
        # serpentine tile ids: 0..T-1, T-1..0, ...
        idx = np.arange(NPC)
        rnd, off = divmod(idx, TPC)
        tile_ids = np.where(rnd % 2 == 0, off, TPC - 1 - off)
        slot_in_tile = rnd
        pos = tile_ids * 128 + slot_in_tile
        pos_local[v0 + order] = pos

    g_pos = (np.arange(N_NODES) // NPC) * NPAD + pos_local  # table row per node

    # per-core per-tile edge buckets
    kd = col // NPC
    src_pos = g_pos[row]
    dst_slot = pos_local[col]
    dst_tile = dst_slot // 128
    dst_loc = dst_slot % 128
    is_lo = src_pos < LO_CAP

    # counts to size K_lo / K_hi uniformly across the SPMD program
    tile_key = kd * TPC + dst_tile
    n_lo = np.bincount(tile_key[is_lo], minlength=M_CORES * TPC)
    n_hi = np.bincount(tile_key[~is_lo], minlength=M_CORES * TPC)
    K_lo = max(1, int(math.ceil(n_lo.max() / 128)))
    K_hi = max(1, int(math.ceil(n_hi.max() / 128)))
    K = K_lo + K_hi
    NCH = TPC * K

    per_core = []
    for k in range(M_CORES):
        idx16 = np.zeros((TPC, K, 128), dtype=np.int16)    # pad -> row 0
        dloc_a = np.full((TPC, K, 128), 200.0, dtype=np.float32)
        m = kd == k
        tl, lc, sp, lo = dst_tile[m], dst_loc[m], src_pos[m], is_lo[m]
        for t in range(TPC):
            mt = tl == t
            for stream, base in ((lo & mt, 0), ((~lo) & mt, K_lo)):
                sps = sp[stream]
                lcs = lc[stream]
                n = sps.size
                if base == 0:
                    vals = sps
                else:
                    vals = sps - HI_OFF
                flat_i = idx16[t].reshape(-1)
                flat_d = dloc_a[t].reshape(-1)
                o = base * 128
                flat_i[o : o + n] = vals.astype(np.int16)
                flat_d[o : o + n] = lcs.astype(np.float32)

        # SWDGE wrapped-16 index layout, replicated across the 8 Q7 groups
        flat = idx16.reshape(-1)
        wrapped = flat.reshape(-1, 16).T                    # [16, NCH*8]
        idx_sb = np.tile(wrapped, (8, 1)).copy()            # [128, NCH*8]
        # per-chunk destination-local column, lane-major
        dloc_sb = dloc_a.transpose(2, 0, 1).reshape(128, NCH).copy()
        # per-slot dinv (0 on dummy slots)
        dv = np.zeros(NPAD, dtype=np.float32)
        v0 = k * NPC
        dv[pos_local[v0 : v0 + NPC]] = dinv[v0 : v0 + NPC]
        dinv_sb = dv.reshape(TPC, 128).T.copy()             # [128, TPC]
        per_core.append(dict(idx=idx_sb, dloc=dloc_sb, dinv=dinv_sb))

    return dict(
        K_lo=K_lo, K_hi=K_hi, NCH=NCH, per_core=per_core,
        pos_local=pos_local, dinv=dinv,
    )


# ---------------------------------------------------------------------------
# device program
# ---------------------------------------------------------------------------
def _build_nc(K_lo: int, K_hi: int, with_bias: bool, reps: int = 0, ablate: str = ''):
    abl = set(a for a in ablate.split(',') if a)
    """Build + compile the SPMD program.

    reps > 0 additionally emits a timing loop: the full pipeline runs once
    (correct, fills the gful tables), then a hardware For_i loop re-runs
    the whole body `reps` times with the collectives elided (collectives
    cannot sit inside control flow) so device time dominates wall clock.
    """
    import concourse.bacc as bacc
    import concourse.mybir as mybir
    from concourse import tile
    from concourse._compat import get_trn_type

    dt = mybir.dt
    K = K_lo + K_hi
    NCH = TPC * K
    NW = NCH * 8

    nc = bacc.Bacc(
        get_trn_type() or "TRN2",
        target_bir_lowering=False,
        debug=False,
        enable_asserts=False,
        num_devices=M_CORES,
    )

    # I/O
    xT_p = nc.dram_tensor("xT", [128, NPAD], dt.float32, kind="ExternalInput")
    W1_p = nc.dram_tensor("W1", [IN_C, HID_C], dt.float32, kind="ExternalInput")
    W2_p = nc.dram_tensor("W2", [HID_C, HID_C], dt.float32, kind="ExternalInput")
    W3_p = nc.dram_tensor("W3", [HID_C, OUT_C], dt.float32, kind="ExternalInput")
    dinv_p = nc.dram_tensor("dinv", [128, TPC], dt.float32, kind="ExternalInput")
    dloc_p = nc.dram_tensor("dloc", [128, NCH], dt.float32, kind="ExternalInput")
    idx_p = nc.dram_tensor("idx", [128, NW], dt.int16, kind="ExternalInput")
    iota_p = nc.dram_tensor("iota", [128, 128], dt.float32, kind="ExternalInput")
    ident_p = nc.dram_tensor("ident", [128, 128], dt.float32, kind="ExternalInput")
    if with_bias:
        b1_p = nc.dram_tensor("b1r", [128, HID_C], dt.float32, kind="ExternalInput")
        b2_p = nc.dram_tensor("b2r", [128, HID_C], dt.float32, kind="ExternalInput")
        b3_p = nc.dram_tensor("b3r", [128, OUT_C], dt.float32, kind="ExternalInput")
    out_p = nc.dram_tensor("out", [NPAD, OUT_C], dt.float32, kind="ExternalOutput")

    RG = [list(range(M_CORES))]
    AF = mybir.ActivationFunctionType
    OP = mybir.AluOpType

    with tile.TileContext(nc) as tc, tc.tile_pool(name="persist", bufs=1) as pp:
        # persistent SBUF tiles (one slot each)
        hT_a = pp.tile([128, NPAD], dt.float32, name="hT_a")
        hT_b = pp.tile([128, NPAD], dt.float32, name="hT_b")
        w1_sb = pp.tile([128, HID_C], dt.float32, name="w1_sb")
        w2_sb = pp.tile([128, HID_C], dt.float32, name="w2_sb")
        w3_sb = pp.tile([128, OUT_C], dt.float32, name="w3_sb")
        dinv_sb = pp.tile([128, TPC], dt.float32, name="dinv_sb")
        dloc_sb = pp.tile([128, NCH], dt.float32, name="dloc_sb")
        idx_sb = pp.tile([128, NW], dt.int16, name="idx_sb")
        iota_sb = pp.tile([128, 128], dt.float32, name="iota_sb")
        ident_sb = pp.tile([128, 128], dt.float32, name="ident_sb")
        bias_sb = []

        nc.sync.dma_start(hT_a[:], xT_p[:])
        nc.sync.dma_start(w1_sb[:], W1_p[:])
        nc.sync.dma_start(w2_sb[:], W2_p[:])
        nc.sync.dma_start(w3_sb[:], W3_p[:])
        nc.sync.dma_start(dinv_sb[:], dinv_p[:])
        nc.sync.dma_start(dloc_sb[:], dloc_p[:])
        nc.sync.dma_start(idx_sb[:], idx_p[:])
        nc.sync.dma_start(iota_sb[:], iota_p[:])
        nc.sync.dma_start(ident_sb[:], ident_p[:])
        if with_bias:
            for p, cc in ((b1_p, HID_C), (b2_p, HID_C), (b3_p, OUT_C)):
                t = pp.tile([128, cc], dt.float32, name=f"bias{len(bias_sb)}_sb")
                nc.sync.dma_start(t[:], p[:])
                bias_sb.append(t)

        layers = [
            (w1_sb, HID_C, True, hT_a, hT_b),
            (w2_sb, HID_C, True, hT_b, hT_a),
            (w3_sb, OUT_C, False, hT_a, None),
        ]

        with (
            tc.tile_pool(name="gsb", bufs=2) as gsb_pool,
            tc.tile_pool(name="msg", bufs=4) as msg_pool,
            tc.tile_pool(name="oh", bufs=4) as oh_pool,
            tc.tile_pool(name="eps", bufs=3) as eps_pool,
            tc.tile_pool(name="psA", bufs=2, space="PSUM") as psA_pool,
            tc.tile_pool(name="psS", bufs=2, space="PSUM") as psS_pool,
            tc.tile_pool(name="psT", bufs=2, space="PSUM") as psT_pool,
            tc.tile_pool(name="dram", bufs=1, space="DRAM") as dram_pool,
        ):
            glocs = [
                dram_pool.tile([NPAD, c], dt.float32, name=f"gloc{i}")
                for i, c in enumerate([HID_C, HID_C, OUT_C])
            ]
            gfuls = [
                dram_pool.tile(
                    [TBL, c], dt.float32,
                    addr_space="Shared" if M_CORES > 4 else "Local",
                    name=f"gful{i}",
                )
                for i, c in enumerate([HID_C, HID_C, OUT_C])
            ]

            def emit_layers(with_cc):
                for li, (w_sb, C, relu, hT_in, hT_out) in enumerate(layers):
                    gloc, gful = glocs[li], gfuls[li]

                    # stage A: g = dinv * (h @ W) for own nodes
                    g_sb = gsb_pool.tile([128, TPC, C], dt.float32, tag="gsb")
                    for t in range(TPC):
                        psA = psA_pool.tile([128, C], dt.float32, tag="psA")
                        nc.tensor.matmul(
                            psA[:],
                            lhsT=hT_in[:, t * 128 : (t + 1) * 128],
                            rhs=w_sb[:, :C],
                            start=True,
                            stop=True,
                        )
                        nc.vector.tensor_scalar_mul(
                            g_sb[:, t, :], psA[:], dinv_sb[:, t : t + 1]
                        )
                    nc.sync.dma_start(
                        gloc[:].rearrange("(t p) c -> p t c", p=128), g_sb[:]
                    )

                    # stage B: replicate the g table
                    if with_cc:
                        nc.gpsimd.collective_compute(
                            "AllGather",
                            OP.bypass,
                            replica_groups=RG,
                            ins=[gloc[:]],
                            outs=[gful[:]],
                        )
                    g_lo = gful[0 : min(LO_CAP, TBL), :]
                    g_hi = gful[HI_OFF:TBL, :]

                    # stage C: gather + one-hot scatter per destination tile
                    Ce = 64 if 'e64' in abl else C
                    for t in range(TPC):
                        woff = t * K * 8
                        if 'nogather' not in abl:
                            msg = msg_pool.tile([128, K, Ce], dt.float32, tag="msg")
                            nc.gpsimd.dma_gather(
                                msg[:, 0:K_lo, :],
                                g_lo[:, :Ce],
                                idx_sb[:, woff : woff + K_lo * 8],
                                K_lo * 128,
                                K_lo * 128,
                                Ce,
                                elem_step=C,
                                single_packet=False,
                            )
                            nc.gpsimd.dma_gather(
                                msg[:, K_lo:K, :],
                                g_hi[:, :Ce],
                                idx_sb[:, woff + K_lo * 8 : woff + K * 8],
                                K_hi * 128,
                                K_hi * 128,
                                Ce,
                                elem_step=C,
                                single_packet=False,
                            )
                        psS = psS_pool.tile([128, Ce], dt.float32, tag="psS")
                        n_mm = 1 if 'nomm' in abl else K
                        for j in range(n_mm):
                            if 'nooh' in abl:
                                mm_lhs = ident_sb
                            else:
                                oh = oh_pool.tile([128, 128], dt.float32, tag="oh")
                                nc.vector.tensor_scalar(
                                    oh[:],
                                    iota_sb[:],
                                    dloc_sb[:, t * K + j : t * K + j + 1],
                                    None,
                                    op0=OP.is_equal,
                                )
                                mm_lhs = oh
                            mm_rhs = (
                                msg[:, j, :]
                                if 'nogather' not in abl
                                else hT_a[:, j * 64 : j * 64 + Ce]
                            )
                            nc.tensor.matmul(
                                psS[:],
                                lhsT=mm_lhs[:],
                                rhs=mm_rhs,
                                start=(j == 0),
                                stop=(j == n_mm - 1),
                            )
                        # epilogue: + self-loop, * dinv, (+bias), relu
                        acc = eps_pool.tile([128, C], dt.float32, tag="acc")
                        nc.vector.tensor_add(
                            acc[:, :Ce], psS[:], g_sb[:, t, :Ce]
                        )
                        if Ce < C:
                            nc.vector.tensor_copy(acc[:, Ce:], g_sb[:, t, Ce:])
                        h_t = eps_pool.tile([128, C], dt.float32, tag="h_t")
                        if with_bias:
                            nc.vector.tensor_scalar_mul(
                                acc[:], acc[:], dinv_sb[:, t : t + 1]
                            )
                            nc.vector.tensor_add(acc[:], acc[:], bias_sb[li][:])
                            if relu:
                                nc.scalar.activation(h_t[:], acc[:], AF.Relu)
                            else:
                                nc.scalar.copy(h_t[:], acc[:])
                        else:
                            if relu:
                                nc.scalar.activation(
                                    h_t[:], acc[:], AF.Relu,
                                    scale=dinv_sb[:, t : t + 1],
                                )
                            else:
                                nc.scalar.mul(
                                    h_t[:], acc[:], dinv_sb[:, t : t + 1]
                                )
                        if hT_out is not None:
                            psT = psT_pool.tile([128, 128], dt.float32, tag="psT")
                            nc.tensor.transpose(psT[:], h_t[:], ident_sb[:])
                            nc.vector.tensor_copy(
                                hT_out[:, t * 128 : (t + 1) * 128], psT[:]
                            )
                        else:
                            nc.sync.dma_start(
                                out_p[t * 128 : (t + 1) * 128, :], h_t[:]
                            )

            emit_layers(with_cc=True)
            if reps:
                with tc.For_i(0, reps, 1):
                    emit_layers(with_cc=False)

    nc.compile()
    return nc


_NC_CACHE: dict = {}


def _get_nc(K_lo, K_hi, with_bias):
    key = (K_lo, K_hi, with_bias)
    if key not in _NC_CACHE:
        _NC_CACHE[key] = _build_nc(K_lo, K_hi, with_bias)
    return _NC_CACHE[key]


# ---------------------------------------------------------------------------
# entry point
# ---------------------------------------------------------------------------
def _prepare(x, edge_index, W1, b1, W2, b2, W3, b3):
    x = np.asarray(x, dtype=np.float32)
    W1 = np.asarray(W1, dtype=np.float32)
    W2 = np.asarray(W2, dtype=np.float32)
    W3 = np.asarray(W3, dtype=np.float32)
    b1 = np.asarray(b1, dtype=np.float32)
    b2 = np.asarray(b2, dtype=np.float32)
    b3 = np.asarray(b3, dtype=np.float32)

    plan = _plan(np.asarray(edge_index))
    with_bias = bool(np.any(b1) or np.any(b2) or np.any(b3))
    nc = _get_nc(plan["K_lo"], plan["K_hi"], with_bias)

    iota = np.tile(np.arange(128, dtype=np.float32), (128, 1))
    ident = np.eye(128, dtype=np.float32)
    pos_local = plan["pos_local"]

    in_maps = []
    for k in range(M_CORES):
        pc = plan["per_core"][k]
        v0 = k * NPC
        xT = np.zeros((128, NPAD), dtype=np.float32)
        xT[:, pos_local[v0 : v0 + NPC]] = x[v0 : v0 + NPC].T
        im = dict(
            xT=xT, W1=W1, W2=W2, W3=W3,
            dinv=pc["dinv"], dloc=pc["dloc"], idx=pc["idx"],
            iota=iota, ident=ident,
        )
        if with_bias:
            im["b1r"] = np.tile(b1, (128, 1)).astype(np.float32)
            im["b2r"] = np.tile(b2, (128, 1)).astype(np.float32)
            im["b3r"] = np.tile(b3, (128, 1)).astype(np.float32)
        in_maps.append(im)

    def unpermute(results):
        out = np.empty((N_NODES, OUT_C), dtype=np.float32)
        for k in range(M_CORES):
            v0 = k * NPC
            r = results[k]["out"]
            out[v0 : v0 + NPC] = r[pos_local[v0 : v0 + NPC]]
        return out

    return nc, in_maps, unpermute


def kernel(x, edge_index, W1, b1, W2, b2, W3, b3):
    from concourse.bass_utils import run_bass_kernel_spmd

    nc, in_maps, unpermute = _prepare(x, edge_index, W1, b1, W2, b2, W3, b3)
    res = run_bass_kernel_spmd(nc, in_maps, list(range(M_CORES)))
    return unpermute(res.results)



# revision 7
# speedup vs baseline: 1.0388x; 1.0388x over previous
"""3-layer GCN node classifier on 8 Trainium2 NeuronCores.

Math (per layer, folding the symmetric normalization):
    deg[v]  = in-degree(v) (with self loop), dinv = rsqrt(deg)
    g       = dinv * (h @ W)                  (rows scaled)
    out[c]  = dinv[c] * ( sum_{e: col=c} g[row_e] + g[c] ) + b
    h_next  = relu(out)      (layers 1,2; layer 3 has no relu)

Distribution: nodes are range-sharded across 8 cores (graph parallel).
Each core computes g for its own nodes (dense matmul), the g-table is
all-gathered (bf16) to every core's HBM, each core then gathers the
rows for the edges whose *destination* it owns (SWDGE dma_gather,
batched over tile groups) and scatter-adds them into per-destination-
tile PSUM accumulators via one-hot bf16 matmuls on the TensorEngine.

Precision: h stays fp32 (stage-A matmuls fp32); only the replicated
message table g is rounded to bf16, and the scatter accumulation runs
in fp32 PSUM, so per-layer error is a single bf16 rounding (~0.4% el).

Host-side preprocessing only touches edge_index (graph structure):
CSR-style bucketing of edges by destination tile, degree computation,
a within-core node permutation that load-balances destination tiles,
and int16 gather-index packing (the 50k-row table is split into two
overlapping <=32768-row views because SWDGE gather indices are int16).
"""

import math
import os
import numpy as np

# ---------------------------------------------------------------------------
# problem constants (hardcoded per contract; kernel.py must be self-contained)
# ---------------------------------------------------------------------------
N_NODES = 50000
IN_C, HID_C, OUT_C = 128, 128, 64
M_CORES = 8
NPC = N_NODES // M_CORES            # 6250 nodes per core
TPC = (NPC + 127) // 128            # 49 destination tiles per core
NPAD = TPC * 128                    # 6272 padded nodes per core
TBL = M_CORES * NPAD                # 50176 rows in the all-gathered table
LO_CAP = 32768                      # int16 index reach
HI_OFF = max(0, TBL - 32768)        # 17408: hi view = table[HI_OFF:]
GRP = 10                            # dest tiles per batched SWDGE gather

F32 = "float32"


def _tile_groups():
    """[(tile_start, tile_count), ...] covering 0..TPC in GRP chunks."""
    return [(t0, min(GRP, TPC - t0)) for t0 in range(0, TPC, GRP)]


# ---------------------------------------------------------------------------
# host-side graph preprocessing (indices only)
# ---------------------------------------------------------------------------
def _plan(edge_index: np.ndarray):
    """Build per-core index/metadata arrays from edge_index [2, E]."""
    row = np.asarray(edge_index[0], dtype=np.int64)
    col = np.asarray(edge_index[1], dtype=np.int64)

    deg_in = np.bincount(col, minlength=N_NODES)          # edges only
    dinv = 1.0 / np.sqrt(deg_in + 1.0)                     # + self loop

    # within-core permutation: snake-fill tiles with degree-sorted nodes so
    # every destination tile carries a near-equal number of incoming edges.
    pos_local = np.empty(N_NODES, dtype=np.int64)
    for k in range(M_CORES):
        v0 = k * NPC
        d = deg_in[v0 : v0 + NPC]
        order = np.argsort(-d, kind="stable")              # heavy first
        # serpentine tile ids: 0..T-1, T-1..0, ...
        idx = np.arange(NPC)
        rnd, off = divmod(idx, TPC)
        tile_ids = np.where(rnd % 2 == 0, off, TPC - 1 - off)
        slot_in_tile = rnd
        pos = tile_ids * 128 + slot_in_tile
        pos_local[v0 + order] = pos

    g_pos = (np.arange(N_NODES) // NPC) * NPAD + pos_local  # table row per node

    # per-core per-tile edge buckets
    kd = col // NPC
    src_pos = g_pos[row]
    dst_slot = pos_local[col]
    dst_tile = dst_slot // 128
    dst_loc = dst_slot % 128
    is_lo = src_pos < LO_CAP

    # counts to size K_lo / K_hi uniformly across the SPMD program
    tile_key = kd * TPC + dst_tile
    n_lo = np.bincount(tile_key[is_lo], minlength=M_CORES * TPC)
    n_hi = np.bincount(tile_key[~is_lo], minlength=M_CORES * TPC)
    K_lo = max(1, int(math.ceil(n_lo.max() / 128)))
    K_hi = max(1, int(math.ceil(n_hi.max() / 128)))
    K = K_lo + K_hi
    NCH = TPC * K

    groups = _tile_groups()

    per_core = []
    for k in range(M_CORES):
        idx_lo = np.zeros((TPC, K_lo, 128), dtype=np.int16)  # pad -> row 0
        idx_hi = np.zeros((TPC, K_hi, 128), dtype=np.int16)
        dloc_a = np.full((TPC, K, 128), 200.0, dtype=np.float32)
        m = kd == k
        tl, lc, sp, lo = dst_tile[m], dst_loc[m], src_pos[m], is_lo[m]
        for t in range(TPC):
            mt = tl == t
            for stream, idx16, base in ((lo & mt, idx_lo, 0),
                                        ((~lo) & mt, idx_hi, K_lo)):
                sps = sp[stream]
                lcs = lc[stream]
                n = sps.size
                vals = sps if base == 0 else sps - HI_OFF
                idx16[t].reshape(-1)[:n] = vals.astype(np.int16)
                flat_d = dloc_a[t].reshape(-1)
                o = base * 128
                flat_d[o : o + n] = lcs.astype(np.float32)

        # batched-call slot order: per tile group, all lo chunks of its
        # tiles, then all hi chunks.  SWDGE wrapped-16 layout, replicated
        # across the 8 Q7 groups.
        parts = []
        for t0, gn in groups:
            parts.append(idx_lo[t0 : t0 + gn].reshape(-1))
            parts.append(idx_hi[t0 : t0 + gn].reshape(-1))
        flat = np.concatenate(parts)
        wrapped = flat.reshape(-1, 16).T                    # [16, slots/16]
        idx_sb = np.tile(wrapped, (8, 1)).copy()            # [128, slots/16]
        # per-chunk destination-local column, lane-major
        dloc_sb = dloc_a.transpose(2, 0, 1).reshape(128, NCH).copy()
        # per-slot dinv (0 on dummy slots)
        dv = np.zeros(NPAD, dtype=np.float32)
        v0 = k * NPC
        dv[pos_local[v0 : v0 + NPC]] = dinv[v0 : v0 + NPC]
        dinv_sb = dv.reshape(TPC, 128).T.copy()             # [128, TPC]
        per_core.append(dict(idx=idx_sb, dloc=dloc_sb, dinv=dinv_sb))

    return dict(
        K_lo=K_lo, K_hi=K_hi, NCH=NCH, per_core=per_core,
        pos_local=pos_local, dinv=dinv,
    )


# ---------------------------------------------------------------------------
# device program
# ---------------------------------------------------------------------------
def _build_nc(K_lo: int, K_hi: int, with_bias: bool, reps: int = 0, ablate: str = ''):
    """Build + compile the SPMD program.

    reps > 0 additionally emits a timing loop: the full pipeline runs once
    (correct, fills the gful tables), then a hardware For_i loop re-runs
    the whole body `reps` times with the collectives elided (collectives
    cannot sit inside control flow) so device time dominates wall clock.
    """
    import concourse.bacc as bacc
    import concourse.mybir as mybir
    from concourse import tile
    from concourse._compat import get_trn_type

    dt = mybir.dt
    K = K_lo + K_hi
    NCH = TPC * K
    NW = NCH * 8
    groups = _tile_groups()

    nc = bacc.Bacc(
        get_trn_type() or "TRN2",
        target_bir_lowering=False,
        debug=False,
        enable_asserts=False,
        num_devices=M_CORES,
    )

    # I/O
    xT_p = nc.dram_tensor("xT", [128, NPAD], dt.float32, kind="ExternalInput")
    W1_p = nc.dram_tensor("W1", [IN_C, HID_C], dt.float32, kind="ExternalInput")
    W2_p = nc.dram_tensor("W2", [HID_C, HID_C], dt.float32, kind="ExternalInput")
    W3_p = nc.dram_tensor("W3", [HID_C, OUT_C], dt.float32, kind="ExternalInput")
    dinv_p = nc.dram_tensor("dinv", [128, TPC], dt.float32, kind="ExternalInput")
    dloc_p = nc.dram_tensor("dloc", [128, NCH], dt.float32, kind="ExternalInput")
    idx_p = nc.dram_tensor("idx", [128, NW], dt.int16, kind="ExternalInput")
    iota_p = nc.dram_tensor("iota", [128, 128], dt.float32, kind="ExternalInput")
    ident_p = nc.dram_tensor("ident", [128, 128], dt.float32, kind="ExternalInput")
    if with_bias:
        b1_p = nc.dram_tensor("b1r", [128, HID_C], dt.float32, kind="ExternalInput")
        b2_p = nc.dram_tensor("b2r", [128, HID_C], dt.float32, kind="ExternalInput")
        b3_p = nc.dram_tensor("b3r", [128, OUT_C], dt.float32, kind="ExternalInput")
    out_p = nc.dram_tensor("out", [NPAD, OUT_C], dt.float32, kind="ExternalOutput")

    RG = [list(range(M_CORES))]
    AF = mybir.ActivationFunctionType
    OP = mybir.AluOpType

    with tile.TileContext(nc) as tc, tc.tile_pool(name="persist", bufs=1) as pp:
        # persistent SBUF tiles (one slot each)
        hT_a = pp.tile([128, NPAD], dt.float32, name="hT_a")
        hT_b = pp.tile([128, NPAD], dt.float32, name="hT_b")
        w1_sb = pp.tile([128, HID_C], dt.float32, name="w1_sb")
        w2_sb = pp.tile([128, HID_C], dt.float32, name="w2_sb")
        w3_sb = pp.tile([128, OUT_C], dt.float32, name="w3_sb")
        dinv_sb = pp.tile([128, TPC], dt.float32, name="dinv_sb")
        dloc_sb = pp.tile([128, NCH], dt.float32, name="dloc_sb")
        idx_sb = pp.tile([128, NW], dt.int16, name="idx_sb")
        iota_sb = pp.tile([128, 128], dt.bfloat16, name="iota_sb")
        ident_sb = pp.tile([128, 128], dt.float32, name="ident_sb")
        g16_sb = pp.tile([128, TPC, HID_C], dt.bfloat16, name="g16_sb")
        bias_sb = []

        nc.sync.dma_start(hT_a[:], xT_p[:])
        nc.sync.dma_start(w1_sb[:], W1_p[:])
        nc.sync.dma_start(w2_sb[:], W2_p[:])
        nc.sync.dma_start(w3_sb[:], W3_p[:])
        nc.sync.dma_start(dinv_sb[:], dinv_p[:])
        nc.sync.dma_start(dloc_sb[:], dloc_p[:])
        nc.sync.dma_start(idx_sb[:], idx_p[:])
        if with_bias:
            for p, cc in ((b1_p, HID_C), (b2_p, HID_C), (b3_p, OUT_C)):
                t = pp.tile([128, cc], dt.float32, name=f"bias{len(bias_sb)}_sb")
                nc.sync.dma_start(t[:], p[:])
                bias_sb.append(t)

        with tc.tile_pool(name="cvt", bufs=1) as cvt_pool:
            iota32 = cvt_pool.tile([128, 128], dt.float32, tag="iota32")
            nc.sync.dma_start(iota32[:], iota_p[:])
            nc.sync.dma_start(ident_sb[:], ident_p[:])
            nc.vector.tensor_copy(iota_sb[:], iota32[:])

        layers = [
            (w1_sb, HID_C, True, hT_a, hT_b),
            (w2_sb, HID_C, True, hT_b, hT_a),
            (w3_sb, OUT_C, False, hT_a, None),
        ]

        with (
            tc.tile_pool(name="mlo", bufs=2) as mlo_pool,
            tc.tile_pool(name="mhi", bufs=2) as mhi_pool,
            tc.tile_pool(name="oh", bufs=4) as oh_pool,
            tc.tile_pool(name="eps", bufs=3) as eps_pool,
            tc.tile_pool(name="psA", bufs=2, space="PSUM") as psA_pool,
            tc.tile_pool(name="psS", bufs=2, space="PSUM") as psS_pool,
            tc.tile_pool(name="psT", bufs=2, space="PSUM") as psT_pool,
            tc.tile_pool(name="dram", bufs=1, space="DRAM") as dram_pool,
        ):
            # All three tables are 128 wide (bf16 rows must be a multiple of
            # 256B for SWDGE).  Layer 3 only uses the first 64 columns; the
            # upper half carries stale layer-2 data that the epilogue never
            # reads.
            CW = HID_C
            glocs = [
                dram_pool.tile([NPAD, CW], dt.bfloat16, name=f"gloc{i}")
                for i in range(3)
            ]
            gfuls = [
                dram_pool.tile(
                    [TBL, CW], dt.bfloat16,
                    addr_space="Shared" if M_CORES > 4 else "Local",
                    name=f"gful{i}",
                )
                for i in range(3)
            ]

            def emit_layers(with_cc):
                for li, (w_sb, C, relu, hT_in, hT_out) in enumerate(layers):
                    gloc, gful = glocs[li], gfuls[li]

                    # stage A: g = dinv * (h @ W) for own nodes, bf16 table
                    for t in range(TPC):
                        psA = psA_pool.tile([128, C], dt.float32, tag="psA")
                        nc.tensor.matmul(
                            psA[:],
                            lhsT=hT_in[:, t * 128 : (t + 1) * 128],
                            rhs=w_sb[:, :C],
                            start=True,
                            stop=True,
                        )
                        # per-node (= per-partition) scale on the idle ACT
                        nc.scalar.mul(
                            g16_sb[:, t, :C], psA[:], dinv_sb[:, t : t + 1]
                        )
                    nc.sync.dma_start(
                        gloc[:].rearrange("(t p) c -> p t c", p=128),
                        g16_sb[:],
                    )

                    # stage B: replicate the g table
                    if with_cc:
                        nc.gpsimd.collective_compute(
                            "AllGather",
                            OP.bypass,
                            replica_groups=RG,
                            ins=[gloc[:]],
                            outs=[gful[:]],
                        )
                    g_lo = gful[0 : min(LO_CAP, TBL), :]
                    g_hi = gful[HI_OFF:TBL, :]

                    # stage C: batched gather + one-hot scatter per dest tile
                    woff = 0
                    for t0, gn in groups:
                        msg_lo = mlo_pool.tile(
                            [128, GRP * K_lo, CW], dt.bfloat16, tag="mlo"
                        )
                        msg_hi = mhi_pool.tile(
                            [128, GRP * K_hi, CW], dt.bfloat16, tag="mhi"
                        )
                        n_lo = gn * K_lo * 128
                        n_hi = gn * K_hi * 128
                        nc.gpsimd.dma_gather(
                            msg_lo[:, 0 : gn * K_lo, :],
                            g_lo[:],
                            idx_sb[:, woff : woff + n_lo // 16],
                            n_lo,
                            n_lo,
                            CW,
                            elem_step=CW,
                            single_packet=False,
                        )
                        woff += n_lo // 16
                        nc.gpsimd.dma_gather(
                            msg_hi[:, 0 : gn * K_hi, :],
                            g_hi[:],
                            idx_sb[:, woff : woff + n_hi // 16],
                            n_hi,
                            n_hi,
                            CW,
                            elem_step=CW,
                            single_packet=False,
                        )
                        woff += n_hi // 16

                        for gi in range(gn):
                            t = t0 + gi
                            psS = psS_pool.tile([128, CW], dt.float32, tag="psS")
                            for j in range(K):
                                oh = oh_pool.tile([128, 128], dt.bfloat16, tag="oh")
                                nc.vector.tensor_scalar(
                                    oh[:],
                                    iota_sb[:],
                                    dloc_sb[:, t * K + j : t * K + j + 1],
                                    None,
                                    op0=OP.is_equal,
                                )
                                if j < K_lo:
                                    mm_rhs = msg_lo[:, gi * K_lo + j, :]
                                else:
                                    mm_rhs = msg_hi[:, gi * K_hi + (j - K_lo), :]
                                nc.tensor.matmul(
                                    psS[:],
                                    lhsT=oh[:],
                                    rhs=mm_rhs,
                                    start=(j == 0),
                                    stop=(j == K - 1),
                                )
                            # epilogue: + self-loop, * dinv, (+bias), relu
                            acc = eps_pool.tile([128, C], dt.float32, tag="acc")
                            nc.vector.tensor_add(
                                acc[:], psS[:, :C], g16_sb[:, t, :C]
                            )
                            h_t = eps_pool.tile([128, C], dt.float32, tag="h_t")
                            if with_bias:
                                nc.vector.tensor_scalar_mul(
                                    acc[:], acc[:], dinv_sb[:, t : t + 1]
                                )
                                nc.vector.tensor_add(acc[:], acc[:], bias_sb[li][:])
                                if relu:
                                    nc.scalar.activation(h_t[:], acc[:], AF.Relu)
                                else:
                                    nc.scalar.copy(h_t[:], acc[:])
                            else:
                                if relu:
                                    nc.scalar.activation(
                                        h_t[:], acc[:], AF.Relu,
                                        scale=dinv_sb[:, t : t + 1],
                                    )
                                else:
                                    nc.scalar.mul(
                                        h_t[:], acc[:], dinv_sb[:, t : t + 1]
                                    )
                            if hT_out is not None:
                                psT = psT_pool.tile([128, 128], dt.float32, tag="psT")
                                nc.tensor.transpose(psT[:], h_t[:], ident_sb[:])
                                nc.vector.tensor_copy(
                                    hT_out[:, t * 128 : (t + 1) * 128], psT[:]
                                )
                            else:
                                nc.sync.dma_start(
                                    out_p[t * 128 : (t + 1) * 128, :], h_t[:]
                                )

            emit_layers(with_cc=True)
            if reps:
                with tc.For_i(0, reps, 1):
                    emit_layers(with_cc=False)

    nc.compile()
    return nc


_NC_CACHE: dict = {}


def _get_nc(K_lo, K_hi, with_bias):
    key = (K_lo, K_hi, with_bias)
    if key not in _NC_CACHE:
        _NC_CACHE[key] = _build_nc(K_lo, K_hi, with_bias)
    return _NC_CACHE[key]


# ---------------------------------------------------------------------------
# entry point
# ---------------------------------------------------------------------------
def _prepare(x, edge_index, W1, b1, W2, b2, W3, b3):
    x = np.asarray(x, dtype=np.float32)
    W1 = np.asarray(W1, dtype=np.float32)
    W2 = np.asarray(W2, dtype=np.float32)
    W3 = np.asarray(W3, dtype=np.float32)
    b1 = np.asarray(b1, dtype=np.float32)
    b2 = np.asarray(b2, dtype=np.float32)
    b3 = np.asarray(b3, dtype=np.float32)

    plan = _plan(np.asarray(edge_index))
    with_bias = bool(np.any(b1) or np.any(b2) or np.any(b3))
    nc = _get_nc(plan["K_lo"], plan["K_hi"], with_bias)

    iota = np.tile(np.arange(128, dtype=np.float32), (128, 1))
    ident = np.eye(128, dtype=np.float32)
    pos_local = plan["pos_local"]

    in_maps = []
    for k in range(M_CORES):
        pc = plan["per_core"][k]
        v0 = k * NPC
        xT = np.zeros((128, NPAD), dtype=np.float32)
        xT[:, pos_local[v0 : v0 + NPC]] = x[v0 : v0 + NPC].T
        im = dict(
            xT=xT, W1=W1, W2=W2, W3=W3,
            dinv=pc["dinv"], dloc=pc["dloc"], idx=pc["idx"],
            iota=iota, ident=ident,
        )
        if with_bias:
            im["b1r"] = np.tile(b1, (128, 1)).astype(np.float32)
            im["b2r"] = np.tile(b2, (128, 1)).astype(np.float32)
            im["b3r"] = np.tile(b3, (128, 1)).astype(np.float32)
        in_maps.append(im)

    def unpermute(results):
        out = np.empty((N_NODES, OUT_C), dtype=np.float32)
        for k in range(M_CORES):
            v0 = k * NPC
            r = results[k]["out"]
            out[v0 : v0 + NPC] = r[pos_local[v0 : v0 + NPC]]
        return out

    return nc, in_maps, unpermute


def kernel(x, edge_index, W1, b1, W2, b2, W3, b3):
    from concourse.bass_utils import run_bass_kernel_spmd

    nc, in_maps, unpermute = _prepare(x, edge_index, W1, b1, W2, b2, W3, b3)
    res = run_bass_kernel_spmd(nc, in_maps, list(range(M_CORES)))
    return unpermute(res.results)


# revision 17
# speedup vs baseline: 1.7322x; 1.6675x over previous
"""3-layer GCN node classifier on 8 Trainium2 NeuronCores.

Math (per layer, folding the symmetric normalization):
    deg[v]  = in-degree(v) (with self loop), dinv = rsqrt(deg)
    g       = dinv * (h @ W)                  (rows scaled)
    out[c]  = dinv[c] * ( sum_{e: col=c} g[row_e] + g[c] ) + b
    h_next  = relu(out)      (layers 1,2; layer 3 has no relu)

Distribution: nodes are range-sharded across 8 cores (graph parallel).
Each core computes g for its own nodes (dense matmul), the g-table is
all-gathered (bf16) to every core's HBM, each core then gathers the
rows for the edges whose *destination* it owns (SWDGE dma_gather,
batched over tile groups) and scatter-adds them into per-destination-
tile PSUM accumulators via one-hot bf16 matmuls on the TensorEngine.

Precision: h stays fp32 (stage-A matmuls fp32); only the replicated
message table g is rounded to bf16, and the scatter accumulation runs
in fp32 PSUM, so per-layer error is a single bf16 rounding (~0.4% el).

Host-side preprocessing only touches edge_index (graph structure):
CSR-style bucketing of edges by destination tile, degree computation,
a within-core node permutation that load-balances destination tiles,
and int16 gather-index packing (the 50k-row table is split into two
overlapping <=32768-row views because SWDGE gather indices are int16).
"""

import math
import os
import numpy as np

# ---------------------------------------------------------------------------
# problem constants (hardcoded per contract; kernel.py must be self-contained)
# ---------------------------------------------------------------------------
N_NODES = 50000
IN_C, HID_C, OUT_C = 128, 128, 64
M_CORES = 8
NPC = N_NODES // M_CORES            # 6250 nodes per core
TPC = (NPC + 127) // 128            # 49 destination tiles per core
NPAD = TPC * 128                    # 6272 padded nodes per core
TBL = M_CORES * NPAD                # 50176 rows in the all-gathered table
LO_CAP = 32768                      # int16 index reach
HI_OFF = max(0, TBL - 32768)        # 17408: hi view = table[HI_OFF:]
GRP = 10                            # dest tiles per batched SWDGE gather

F32 = "float32"


def _tile_groups():
    """[(tile_start, tile_count), ...] covering 0..TPC in GRP chunks."""
    return [(t0, min(GRP, TPC - t0)) for t0 in range(0, TPC, GRP)]


# ---------------------------------------------------------------------------
# host-side graph preprocessing (indices only)
# ---------------------------------------------------------------------------
def _plan(edge_index: np.ndarray):
    """Build per-core index/metadata arrays from edge_index [2, E]."""
    row = np.asarray(edge_index[0], dtype=np.int64)
    col = np.asarray(edge_index[1], dtype=np.int64)

    deg_in = np.bincount(col, minlength=N_NODES)          # edges only
    dinv = 1.0 / np.sqrt(deg_in + 1.0)                     # + self loop

    # within-core permutation: snake-fill tiles with degree-sorted nodes so
    # every destination tile carries a near-equal number of incoming edges.
    pos_local = np.empty(N_NODES, dtype=np.int64)
    for k in range(M_CORES):
        v0 = k * NPC
        d = deg_in[v0 : v0 + NPC]
        order = np.argsort(-d, kind="stable")              # heavy first
        # serpentine tile ids: 0..T-1, T-1..0, ...
        idx = np.arange(NPC)
        rnd, off = divmod(idx, TPC)
        tile_ids = np.where(rnd % 2 == 0, off, TPC - 1 - off)
        slot_in_tile = rnd
        pos = tile_ids * 128 + slot_in_tile
        pos_local[v0 + order] = pos

    g_pos = (np.arange(N_NODES) // NPC) * NPAD + pos_local  # table row per node

    # per-core per-tile edge buckets
    kd = col // NPC
    src_pos = g_pos[row]
    dst_slot = pos_local[col]
    dst_tile = dst_slot // 128
    dst_loc = dst_slot % 128
    is_lo = src_pos < LO_CAP

    # counts to size K_lo / K_hi uniformly across the SPMD program
    tile_key = kd * TPC + dst_tile
    n_lo = np.bincount(tile_key[is_lo], minlength=M_CORES * TPC)
    n_hi = np.bincount(tile_key[~is_lo], minlength=M_CORES * TPC)
    K_lo = max(1, int(math.ceil(n_lo.max() / 128)))
    K_hi = max(1, int(math.ceil(n_hi.max() / 128)))
    K = K_lo + K_hi
    NCH = TPC * K

    groups = _tile_groups()

    per_core = []
    for k in range(M_CORES):
        idx_lo = np.zeros((TPC, K_lo, 128), dtype=np.int16)  # pad -> row 0
        idx_hi = np.zeros((TPC, K_hi, 128), dtype=np.int16)
        dloc_a = np.full((TPC, K, 128), 200.0, dtype=np.float32)
        m = kd == k
        tl, lc, sp, lo = dst_tile[m], dst_loc[m], src_pos[m], is_lo[m]
        for t in range(TPC):
            mt = tl == t
            for stream, idx16, base in ((lo & mt, idx_lo, 0),
                                        ((~lo) & mt, idx_hi, K_lo)):
                sps = sp[stream]
                lcs = lc[stream]
                n = sps.size
                vals = sps if base == 0 else sps - HI_OFF
                idx16[t].reshape(-1)[:n] = vals.astype(np.int16)
                flat_d = dloc_a[t].reshape(-1)
                o = base * 128
                flat_d[o : o + n] = lcs.astype(np.float32)

        # batched-call slot order: per tile group, all lo chunks of its
        # tiles, then all hi chunks.  SWDGE wrapped-16 layout, replicated
        # across the 8 Q7 groups.
        parts = []
        for t0, gn in groups:
            parts.append(idx_lo[t0 : t0 + gn].reshape(-1))
            parts.append(idx_hi[t0 : t0 + gn].reshape(-1))
        flat = np.concatenate(parts)
        wrapped = flat.reshape(-1, 16).T                    # [16, slots/16]
        idx_sb = np.tile(wrapped, (8, 1)).copy()            # [128, slots/16]
        # per-chunk destination-local column, lane-major
        dloc_sb = dloc_a.transpose(2, 0, 1).reshape(128, NCH).copy()
        # per-slot dinv (0 on dummy slots)
        dv = np.zeros(NPAD, dtype=np.float32)
        v0 = k * NPC
        dv[pos_local[v0 : v0 + NPC]] = dinv[v0 : v0 + NPC]
        dinv_sb = dv.reshape(TPC, 128).T.copy()             # [128, TPC]
        per_core.append(dict(idx=idx_sb, dloc=dloc_sb, dinv=dinv_sb))

    return dict(
        K_lo=K_lo, K_hi=K_hi, NCH=NCH, per_core=per_core,
        pos_local=pos_local, dinv=dinv,
    )


# ---------------------------------------------------------------------------
# device program
# ---------------------------------------------------------------------------
def _build_nc(K_lo: int, K_hi: int, with_bias: bool, reps: int = 0, ablate: str = '',
              nqueues: int = 4, grp: int | None = None):
    """Build + compile the SPMD program.

    reps > 0 additionally emits a timing loop: the full pipeline runs once
    (correct, fills the gful tables), then a hardware For_i loop re-runs
    the whole body `reps` times with the collectives elided (collectives
    cannot sit inside control flow) so device time dominates wall clock.
    """
    abl = set(a for a in ablate.split(',') if a)
    import concourse.bacc as bacc
    import concourse.mybir as mybir
    from concourse import tile
    from concourse._compat import get_trn_type

    dt = mybir.dt
    K = K_lo + K_hi
    NCH = TPC * K
    NW = NCH * 8
    g_ = grp or GRP
    groups = [(t0, min(g_, TPC - t0)) for t0 in range(0, TPC, g_)]

    nc = bacc.Bacc(
        get_trn_type() or "TRN2",
        target_bir_lowering=False,
        debug=False,
        enable_asserts=False,
        num_devices=M_CORES,
        num_swdge_queues=nqueues,
    )

    # I/O
    xT_p = nc.dram_tensor("xT", [128, NPAD], dt.float32, kind="ExternalInput")
    W1_p = nc.dram_tensor("W1", [IN_C, HID_C], dt.float32, kind="ExternalInput")
    W2_p = nc.dram_tensor("W2", [HID_C, HID_C], dt.float32, kind="ExternalInput")
    W3_p = nc.dram_tensor("W3", [HID_C, OUT_C], dt.float32, kind="ExternalInput")
    dinv_p = nc.dram_tensor("dinv", [128, TPC], dt.float32, kind="ExternalInput")
    dloc_p = nc.dram_tensor("dloc", [128, NCH], dt.float32, kind="ExternalInput")
    idx_p = nc.dram_tensor("idx", [128, NW], dt.int16, kind="ExternalInput")
    iota_p = nc.dram_tensor("iota", [128, 128], dt.float32, kind="ExternalInput")
    ident_p = nc.dram_tensor("ident", [128, 128], dt.float32, kind="ExternalInput")
    if with_bias:
        b1_p = nc.dram_tensor("b1r", [128, HID_C], dt.float32, kind="ExternalInput")
        b2_p = nc.dram_tensor("b2r", [128, HID_C], dt.float32, kind="ExternalInput")
        b3_p = nc.dram_tensor("b3r", [128, OUT_C], dt.float32, kind="ExternalInput")
    out_p = nc.dram_tensor("out", [NPAD, OUT_C], dt.float32, kind="ExternalOutput")

    RG = [list(range(M_CORES))]
    AF = mybir.ActivationFunctionType
    OP = mybir.AluOpType

    with tile.TileContext(nc) as tc, tc.tile_pool(name="persist", bufs=1) as pp:
        # persistent SBUF tiles (one slot each)
        hT_a = pp.tile([128, NPAD], dt.float32, name="hT_a")
        hT_b = pp.tile([128, NPAD], dt.float32, name="hT_b")
        w1_sb = pp.tile([128, HID_C], dt.float32, name="w1_sb")
        w2_sb = pp.tile([128, HID_C], dt.float32, name="w2_sb")
        w3_sb = pp.tile([128, OUT_C], dt.float32, name="w3_sb")
        dinv_sb = pp.tile([128, TPC], dt.float32, name="dinv_sb")
        dloc_sb = pp.tile([128, NCH], dt.float32, name="dloc_sb")
        idx_sb = pp.tile([128, NW], dt.int16, name="idx_sb")
        iota_sb = pp.tile([128, 128], dt.bfloat16, name="iota_sb")
        ident_sb = pp.tile([128, 128], dt.float32, name="ident_sb")
        ident16_sb = pp.tile([128, 128], dt.bfloat16, name="ident16_sb")
        gdt = dt.float32 if 'f32tbl' in abl else dt.bfloat16
        g16_sb = pp.tile([128, TPC, HID_C], gdt, name="g16_sb")
        bias_sb = []

        nc.sync.dma_start(hT_a[:], xT_p[:])
        nc.sync.dma_start(w1_sb[:], W1_p[:])
        nc.sync.dma_start(w2_sb[:], W2_p[:])
        nc.sync.dma_start(w3_sb[:], W3_p[:])
        nc.sync.dma_start(dinv_sb[:], dinv_p[:])
        nc.sync.dma_start(dloc_sb[:], dloc_p[:])
        nc.sync.dma_start(idx_sb[:], idx_p[:])
        if with_bias:
            for p, cc in ((b1_p, HID_C), (b2_p, HID_C), (b3_p, OUT_C)):
                t = pp.tile([128, cc], dt.float32, name=f"bias{len(bias_sb)}_sb")
                nc.sync.dma_start(t[:], p[:])
                bias_sb.append(t)

        with tc.tile_pool(name="cvt", bufs=1) as cvt_pool:
            iota32 = cvt_pool.tile([128, 128], dt.float32, tag="iota32")
            nc.sync.dma_start(iota32[:], iota_p[:])
            nc.sync.dma_start(ident_sb[:], ident_p[:])
            nc.vector.tensor_copy(iota_sb[:], iota32[:])
            nc.vector.tensor_copy(ident16_sb[:], ident_sb[:])

        layers = [
            (w1_sb, HID_C, True, hT_a, hT_b),
            (w2_sb, HID_C, True, hT_b, hT_a),
            (w3_sb, OUT_C, False, hT_a, None),
        ]

        with (
            tc.tile_pool(name="mlo", bufs=2) as mlo_pool,
            tc.tile_pool(name="mhi", bufs=2) as mhi_pool,
            tc.tile_pool(name="oh", bufs=4) as oh_pool,
            tc.tile_pool(name="eps", bufs=3) as eps_pool,
            tc.tile_pool(name="psA", bufs=2, space="PSUM") as psA_pool,
            tc.tile_pool(name="psS", bufs=2, space="PSUM") as psS_pool,
            tc.tile_pool(name="psT", bufs=2, space="PSUM") as psT_pool,
            tc.tile_pool(name="dram", bufs=1, space="DRAM") as dram_pool,
        ):
            # All three tables are 128 wide (bf16 rows must be a multiple of
            # 256B for SWDGE).  Layer 3 only uses the first 64 columns; the
            # upper half carries stale layer-2 data that the epilogue never
            # reads.
            CW = HID_C
            glocs = [
                dram_pool.tile([NPAD, CW], gdt, name=f"gloc{i}")
                for i in range(3)
            ]
            gfuls = [
                dram_pool.tile(
                    [TBL, CW], gdt,
                    addr_space="Shared" if M_CORES > 4 else "Local",
                    name=f"gful{i}",
                )
                for i in range(3)
            ]

            qctr = [0]

            def next_q(ndesc):
                qi = qctr[0] % nqueues
                qctr[0] += 1
                return qi

            def emit_layers(with_cc):
                for li, (w_sb, C, relu, hT_in, hT_out) in enumerate(layers):
                    gloc, gful = glocs[li], gfuls[li]

                    # stage A: g = dinv * (h @ W) for own nodes, bf16 table
                    for t in range(TPC):
                        psA = psA_pool.tile([128, C], dt.float32, tag="psA")
                        nc.tensor.matmul(
                            psA[:],
                            lhsT=hT_in[:, t * 128 : (t + 1) * 128],
                            rhs=w_sb[:, :C],
                            start=True,
                            stop=True,
                        )
                        # per-node (= per-partition) scale on the idle ACT
                        nc.scalar.mul(
                            g16_sb[:, t, :C], psA[:], dinv_sb[:, t : t + 1]
                        )
                    nc.sync.dma_start(
                        gloc[:].rearrange("(t p) c -> p t c", p=128),
                        g16_sb[:],
                    )

                    # stage B: replicate the g table
                    if with_cc:
                        nc.gpsimd.collective_compute(
                            "AllGather",
                            OP.bypass,
                            replica_groups=RG,
                            ins=[gloc[:]],
                            outs=[gful[:]],
                        )
                    g_lo = gful[0 : min(LO_CAP, TBL), :]
                    g_hi = gful[HI_OFF:TBL, :]

                    # stage C: batched gather + one-hot scatter per dest tile
                    woff = 0
                    for t0, gn in groups:
                        msg_lo = mlo_pool.tile(
                            [128, g_ * K_lo, CW], gdt, tag="mlo"
                        )
                        msg_hi = mhi_pool.tile(
                            [128, g_ * K_hi, CW], gdt, tag="mhi"
                        )
                        n_lo = gn * K_lo * 128
                        n_hi = gn * K_hi * 128
                        if 'nogather' not in abl:
                            nc.gpsimd.dma_gather(
                                msg_lo[:, 0 : gn * K_lo, :],
                                g_lo[:],
                                idx_sb[:, woff : woff + n_lo // 16],
                                n_lo,
                                n_lo,
                                CW,
                                elem_step=CW,
                                single_packet=('spkt' in abl),
                                queue_num=next_q(n_lo),
                            )
                        woff += n_lo // 16
                        if 'nogather' not in abl:
                            nc.gpsimd.dma_gather(
                                msg_hi[:, 0 : gn * K_hi, :],
                                g_hi[:],
                                idx_sb[:, woff : woff + n_hi // 16],
                                n_hi,
                                n_hi,
                                CW,
                                elem_step=CW,
                                single_packet=('spkt' in abl),
                                queue_num=next_q(n_hi),
                            )
                        woff += n_hi // 16

                        for gi in range(gn):
                            t = t0 + gi
                            psS = psS_pool.tile([128, CW], dt.float32, tag="psS")
                            n_mm = 1 if 'nomm' in abl else K
                            for j in range(n_mm):
                                if 'nooh' in abl:
                                    mm_lhs = ident_sb if 'f32tbl' in abl else ident16_sb
                                else:
                                    oh = oh_pool.tile([128, 128], dt.bfloat16, tag="oh")
                                    nc.vector.tensor_scalar(
                                        oh[:],
                                        iota_sb[:],
                                        dloc_sb[:, t * K + j : t * K + j + 1],
                                        None,
                                        op0=OP.is_equal,
                                    )
                                    mm_lhs = oh
                                if 'nogather' in abl:
                                    mm_rhs = g16_sb[:, j, :]
                                elif j < K_lo:
                                    mm_rhs = msg_lo[:, gi * K_lo + j, :]
                                else:
                                    mm_rhs = msg_hi[:, gi * K_hi + (j - K_lo), :]
                                nc.tensor.matmul(
                                    psS[:],
                                    lhsT=mm_lhs[:],
                                    rhs=mm_rhs,
                                    start=(j == 0),
                                    stop=(j == n_mm - 1),
                                )
                            # epilogue: + self-loop, * dinv, (+bias), relu
                            acc = eps_pool.tile([128, C], dt.float32, tag="acc")
                            nc.vector.tensor_add(
                                acc[:], psS[:, :C], g16_sb[:, t, :C]
                            )
                            h_t = eps_pool.tile([128, C], dt.float32, tag="h_t")
                            if with_bias:
                                nc.vector.tensor_scalar_mul(
                                    acc[:], acc[:], dinv_sb[:, t : t + 1]
                                )
                                nc.vector.tensor_add(acc[:], acc[:], bias_sb[li][:])
                                if relu:
                                    nc.scalar.activation(h_t[:], acc[:], AF.Relu)
                                else:
                                    nc.scalar.copy(h_t[:], acc[:])
                            else:
                                if relu:
                                    nc.scalar.activation(
                                        h_t[:], acc[:], AF.Relu,
                                        scale=dinv_sb[:, t : t + 1],
                                    )
                                else:
                                    nc.scalar.mul(
                                        h_t[:], acc[:], dinv_sb[:, t : t + 1]
                                    )
                            if hT_out is not None:
                                psT = psT_pool.tile([128, 128], dt.float32, tag="psT")
                                nc.tensor.transpose(psT[:], h_t[:], ident_sb[:])
                                nc.vector.tensor_copy(
                                    hT_out[:, t * 128 : (t + 1) * 128], psT[:]
                                )
                            else:
                                nc.sync.dma_start(
                                    out_p[t * 128 : (t + 1) * 128, :], h_t[:]
                                )

            emit_layers(with_cc=True)
            if reps:
                with tc.For_i(0, reps, 1):
                    emit_layers(with_cc=False)

    nc.compile()
    return nc


_NC_CACHE: dict = {}


def _get_nc(K_lo, K_hi, with_bias):
    key = (K_lo, K_hi, with_bias)
    if key not in _NC_CACHE:
        _NC_CACHE[key] = _build_nc(K_lo, K_hi, with_bias)
    return _NC_CACHE[key]


# ---------------------------------------------------------------------------
# entry point
# ---------------------------------------------------------------------------
def _prepare(x, edge_index, W1, b1, W2, b2, W3, b3):
    x = np.asarray(x, dtype=np.float32)
    W1 = np.asarray(W1, dtype=np.float32)
    W2 = np.asarray(W2, dtype=np.float32)
    W3 = np.asarray(W3, dtype=np.float32)
    b1 = np.asarray(b1, dtype=np.float32)
    b2 = np.asarray(b2, dtype=np.float32)
    b3 = np.asarray(b3, dtype=np.float32)

    plan = _plan(np.asarray(edge_index))
    with_bias = bool(np.any(b1) or np.any(b2) or np.any(b3))
    nc = _get_nc(plan["K_lo"], plan["K_hi"], with_bias)

    iota = np.tile(np.arange(128, dtype=np.float32), (128, 1))
    ident = np.eye(128, dtype=np.float32)
    pos_local = plan["pos_local"]

    in_maps = []
    for k in range(M_CORES):
        pc = plan["per_core"][k]
        v0 = k * NPC
        xT = np.zeros((128, NPAD), dtype=np.float32)
        xT[:, pos_local[v0 : v0 + NPC]] = x[v0 : v0 + NPC].T
        im = dict(
            xT=xT, W1=W1, W2=W2, W3=W3,
            dinv=pc["dinv"], dloc=pc["dloc"], idx=pc["idx"],
            iota=iota, ident=ident,
        )
        if with_bias:
            im["b1r"] = np.tile(b1, (128, 1)).astype(np.float32)
            im["b2r"] = np.tile(b2, (128, 1)).astype(np.float32)
            im["b3r"] = np.tile(b3, (128, 1)).astype(np.float32)
        in_maps.append(im)

    def unpermute(results):
        out = np.empty((N_NODES, OUT_C), dtype=np.float32)
        for k in range(M_CORES):
            v0 = k * NPC
            r = results[k]["out"]
            out[v0 : v0 + NPC] = r[pos_local[v0 : v0 + NPC]]
        return out

    return nc, in_maps, unpermute


def kernel(x, edge_index, W1, b1, W2, b2, W3, b3):
    from concourse.bass_utils import run_bass_kernel_spmd

    nc, in_maps, unpermute = _prepare(x, edge_index, W1, b1, W2, b2, W3, b3)
    res = run_bass_kernel_spmd(nc, in_maps, list(range(M_CORES)))
    return unpermute(res.results)


# revision 19
# speedup vs baseline: 2.8829x; 1.6642x over previous
"""3-layer GCN node classifier on 8 Trainium2 NeuronCores.

Math (per layer, folding the symmetric normalization):
    deg[v]  = in-degree(v) (with self loop), dinv = rsqrt(deg)
    g       = dinv * (h @ W)                  (rows scaled)
    out[c]  = dinv[c] * ( sum_{e: col=c} g[row_e] + g[c] ) + b
    h_next  = relu(out)      (layers 1,2; layer 3 has no relu)

Distribution: nodes are range-sharded across 8 cores (graph parallel).
Each core computes g for its own nodes (dense matmul), the g-table is
all-gathered (bf16) to every core's HBM, each core then gathers the
rows for the edges whose *destination* it owns (SWDGE dma_gather,
batched over tile groups) and scatter-adds them into per-destination-
tile PSUM accumulators via one-hot bf16 matmuls on the TensorEngine.

Precision: h stays fp32 (stage-A matmuls fp32); only the replicated
message table g is rounded to bf16, and the scatter accumulation runs
in fp32 PSUM, so per-layer error is a single bf16 rounding (~0.4% el).

Host-side preprocessing only touches edge_index (graph structure):
CSR-style bucketing of edges by destination tile, degree computation,
a within-core node permutation that load-balances destination tiles,
and int16 gather-index packing (the 50k-row table is split into two
overlapping <=32768-row views because SWDGE gather indices are int16).
"""

import math
import os
import numpy as np

# ---------------------------------------------------------------------------
# problem constants (hardcoded per contract; kernel.py must be self-contained)
# ---------------------------------------------------------------------------
N_NODES = 50000
IN_C, HID_C, OUT_C = 128, 128, 64
M_CORES = 8
NPC = N_NODES // M_CORES            # 6250 nodes per core
TPC = (NPC + 127) // 128            # 49 destination tiles per core
NPAD = TPC * 128                    # 6272 padded nodes per core
TBL = M_CORES * NPAD                # 50176 rows in the all-gathered table
LO_CAP = 32768                      # int16 index reach
HI_OFF = max(0, TBL - 32768)        # 17408: hi view = table[HI_OFF:]
GRP = 10                            # dest tiles per batched SWDGE gather

F32 = "float32"


def _tile_groups():
    """[(tile_start, tile_count), ...] covering 0..TPC in GRP chunks."""
    return [(t0, min(GRP, TPC - t0)) for t0 in range(0, TPC, GRP)]


# ---------------------------------------------------------------------------
# host-side graph preprocessing (indices only)
# ---------------------------------------------------------------------------
def _plan(edge_index: np.ndarray):
    """Build per-core index/metadata arrays from edge_index [2, E]."""
    row = np.asarray(edge_index[0], dtype=np.int64)
    col = np.asarray(edge_index[1], dtype=np.int64)

    deg_in = np.bincount(col, minlength=N_NODES)          # edges only
    dinv = 1.0 / np.sqrt(deg_in + 1.0)                     # + self loop

    # within-core permutation: snake-fill tiles with degree-sorted nodes so
    # every destination tile carries a near-equal number of incoming edges.
    pos_local = np.empty(N_NODES, dtype=np.int64)
    for k in range(M_CORES):
        v0 = k * NPC
        d = deg_in[v0 : v0 + NPC]
        order = np.argsort(-d, kind="stable")              # heavy first
        # serpentine tile ids: 0..T-1, T-1..0, ...
        idx = np.arange(NPC)
        rnd, off = divmod(idx, TPC)
        tile_ids = np.where(rnd % 2 == 0, off, TPC - 1 - off)
        slot_in_tile = rnd
        pos = tile_ids * 128 + slot_in_tile
        pos_local[v0 + order] = pos

    g_pos = (np.arange(N_NODES) // NPC) * NPAD + pos_local  # table row per node

    # per-core per-tile edge buckets
    kd = col // NPC
    src_pos = g_pos[row]
    dst_slot = pos_local[col]
    dst_tile = dst_slot // 128
    dst_loc = dst_slot % 128
    is_lo = src_pos < LO_CAP

    # counts to size K_lo / K_hi uniformly across the SPMD program
    tile_key = kd * TPC + dst_tile
    n_lo = np.bincount(tile_key[is_lo], minlength=M_CORES * TPC)
    n_hi = np.bincount(tile_key[~is_lo], minlength=M_CORES * TPC)
    K_lo = max(1, int(math.ceil(n_lo.max() / 128)))
    K_hi = max(1, int(math.ceil(n_hi.max() / 128)))
    K = K_lo + K_hi
    NCH = TPC * K

    groups = _tile_groups()

    per_core = []
    for k in range(M_CORES):
        idx_lo = np.zeros((TPC, K_lo, 128), dtype=np.int16)  # pad -> row 0
        idx_hi = np.zeros((TPC, K_hi, 128), dtype=np.int16)
        dloc_a = np.full((TPC, K, 128), 200.0, dtype=np.float32)
        m = kd == k
        tl, lc, sp, lo = dst_tile[m], dst_loc[m], src_pos[m], is_lo[m]
        for t in range(TPC):
            mt = tl == t
            for stream, idx16, base in ((lo & mt, idx_lo, 0),
                                        ((~lo) & mt, idx_hi, K_lo)):
                sps = sp[stream]
                lcs = lc[stream]
                n = sps.size
                vals = sps if base == 0 else sps - HI_OFF
                idx16[t].reshape(-1)[:n] = vals.astype(np.int16)
                flat_d = dloc_a[t].reshape(-1)
                o = base * 128
                flat_d[o : o + n] = lcs.astype(np.float32)

        # batched-call slot order: per tile group, all lo chunks of its
        # tiles, then all hi chunks.  SWDGE wrapped-16 layout, replicated
        # across the 8 Q7 groups.
        parts = []
        for t0, gn in groups:
            parts.append(idx_lo[t0 : t0 + gn].reshape(-1))
            parts.append(idx_hi[t0 : t0 + gn].reshape(-1))
        flat = np.concatenate(parts)
        wrapped = flat.reshape(-1, 16).T                    # [16, slots/16]
        idx_sb = np.tile(wrapped, (8, 1)).copy()            # [128, slots/16]
        # per-chunk destination-local column, lane-major
        dloc_sb = dloc_a.transpose(2, 0, 1).reshape(128, NCH).copy()
        # per-slot dinv (0 on dummy slots)
        dv = np.zeros(NPAD, dtype=np.float32)
        v0 = k * NPC
        dv[pos_local[v0 : v0 + NPC]] = dinv[v0 : v0 + NPC]
        dinv_sb = dv.reshape(TPC, 128).T.copy()             # [128, TPC]
        per_core.append(dict(idx=idx_sb, dloc=dloc_sb, dinv=dinv_sb))

    return dict(
        K_lo=K_lo, K_hi=K_hi, NCH=NCH, per_core=per_core,
        pos_local=pos_local, dinv=dinv,
    )


# ---------------------------------------------------------------------------
# device program
# ---------------------------------------------------------------------------
def _build_nc(K_lo: int, K_hi: int, with_bias: bool, reps: int = 0, ablate: str = '',
              nqueues: int = 4, grp: int | None = None):
    """Build + compile the SPMD program.

    reps > 0 additionally emits a timing loop: the full pipeline runs once
    (correct, fills the gful tables), then a hardware For_i loop re-runs
    the whole body `reps` times with the collectives elided (collectives
    cannot sit inside control flow) so device time dominates wall clock.
    """
    abl = set(a for a in ablate.split(',') if a)
    import concourse.bacc as bacc
    import concourse.mybir as mybir
    from concourse import tile
    from concourse._compat import get_trn_type

    dt = mybir.dt
    K = K_lo + K_hi
    NCH = TPC * K
    NW = NCH * 8
    g_ = grp or GRP
    groups = [(t0, min(g_, TPC - t0)) for t0 in range(0, TPC, g_)]

    nc = bacc.Bacc(
        get_trn_type() or "TRN2",
        target_bir_lowering=False,
        debug=False,
        enable_asserts=False,
        num_devices=M_CORES,
        num_swdge_queues=nqueues,
    )

    # I/O
    xT_p = nc.dram_tensor("xT", [128, NPAD], dt.float32, kind="ExternalInput")
    W1_p = nc.dram_tensor("W1", [IN_C, HID_C], dt.float32, kind="ExternalInput")
    W2_p = nc.dram_tensor("W2", [HID_C, HID_C], dt.float32, kind="ExternalInput")
    W3_p = nc.dram_tensor("W3", [HID_C, OUT_C], dt.float32, kind="ExternalInput")
    dinv_p = nc.dram_tensor("dinv", [128, TPC], dt.float32, kind="ExternalInput")
    dloc_p = nc.dram_tensor("dloc", [128, NCH], dt.float32, kind="ExternalInput")
    idx_p = nc.dram_tensor("idx", [128, NW], dt.int16, kind="ExternalInput")
    iota_p = nc.dram_tensor("iota", [128, 128], dt.float32, kind="ExternalInput")
    ident_p = nc.dram_tensor("ident", [128, 128], dt.float32, kind="ExternalInput")
    if with_bias:
        b1_p = nc.dram_tensor("b1r", [128, HID_C], dt.float32, kind="ExternalInput")
        b2_p = nc.dram_tensor("b2r", [128, HID_C], dt.float32, kind="ExternalInput")
        b3_p = nc.dram_tensor("b3r", [128, OUT_C], dt.float32, kind="ExternalInput")
    out_p = nc.dram_tensor("out", [NPAD, OUT_C], dt.float32, kind="ExternalOutput")

    RG = [list(range(M_CORES))]
    AF = mybir.ActivationFunctionType
    OP = mybir.AluOpType

    with tile.TileContext(nc) as tc, tc.tile_pool(name="persist", bufs=1) as pp:
        # persistent SBUF tiles (one slot each)
        hT_a = pp.tile([128, NPAD], dt.float32, name="hT_a")
        hT_b = pp.tile([128, NPAD], dt.float32, name="hT_b")
        w1_sb = pp.tile([128, HID_C], dt.float32, name="w1_sb")
        w2_sb = pp.tile([128, HID_C], dt.float32, name="w2_sb")
        w3_sb = pp.tile([128, OUT_C], dt.float32, name="w3_sb")
        dinv_sb = pp.tile([128, TPC], dt.float32, name="dinv_sb")
        dloc_sb = pp.tile([128, NCH], dt.float32, name="dloc_sb")
        idx_sb = pp.tile([128, NW], dt.int16, name="idx_sb")
        iota_sb = pp.tile([128, 128], dt.bfloat16, name="iota_sb")
        ident_sb = pp.tile([128, 128], dt.float32, name="ident_sb")
        ident16_sb = pp.tile([128, 128], dt.bfloat16, name="ident16_sb")
        gdt = dt.float32 if 'f32tbl' in abl else dt.bfloat16
        g16_sb = pp.tile([128, TPC, HID_C], gdt, name="g16_sb")
        bias_sb = []

        nc.sync.dma_start(hT_a[:], xT_p[:])
        nc.sync.dma_start(w1_sb[:], W1_p[:])
        nc.sync.dma_start(w2_sb[:], W2_p[:])
        nc.sync.dma_start(w3_sb[:], W3_p[:])
        nc.sync.dma_start(dinv_sb[:], dinv_p[:])
        nc.sync.dma_start(dloc_sb[:], dloc_p[:])
        nc.sync.dma_start(idx_sb[:], idx_p[:])
        if with_bias:
            for p, cc in ((b1_p, HID_C), (b2_p, HID_C), (b3_p, OUT_C)):
                t = pp.tile([128, cc], dt.float32, name=f"bias{len(bias_sb)}_sb")
                nc.sync.dma_start(t[:], p[:])
                bias_sb.append(t)

        with tc.tile_pool(name="cvt", bufs=1) as cvt_pool:
            iota32 = cvt_pool.tile([128, 128], dt.float32, tag="iota32")
            nc.sync.dma_start(iota32[:], iota_p[:])
            nc.sync.dma_start(ident_sb[:], ident_p[:])
            nc.vector.tensor_copy(iota_sb[:], iota32[:])
            nc.vector.tensor_copy(ident16_sb[:], ident_sb[:])

        layers = [
            (w1_sb, HID_C, True, hT_a, hT_b),
            (w2_sb, HID_C, True, hT_b, hT_a),
            (w3_sb, OUT_C, False, hT_a, None),
        ]

        mbufs = 3 if 'bufs3' in abl else 2
        with (
            tc.tile_pool(name="mlo", bufs=mbufs) as mlo_pool,
            tc.tile_pool(name="mhi", bufs=mbufs) as mhi_pool,
            tc.tile_pool(name="oh", bufs=4) as oh_pool,
            tc.tile_pool(name="eps", bufs=3) as eps_pool,
            tc.tile_pool(name="psA", bufs=2, space="PSUM") as psA_pool,
            tc.tile_pool(name="psS", bufs=2, space="PSUM") as psS_pool,
            tc.tile_pool(name="psT", bufs=2, space="PSUM") as psT_pool,
            tc.tile_pool(name="dram", bufs=1, space="DRAM") as dram_pool,
        ):
            # All three tables are 128 wide (bf16 rows must be a multiple of
            # 256B for SWDGE).  Layer 3 only uses the first 64 columns; the
            # upper half carries stale layer-2 data that the epilogue never
            # reads.
            CW = HID_C
            glocs = [
                dram_pool.tile([NPAD, CW], gdt, name=f"gloc{i}")
                for i in range(3)
            ]
            gfuls = [
                dram_pool.tile(
                    [TBL, CW], gdt,
                    addr_space="Shared" if M_CORES > 4 else "Local",
                    name=f"gful{i}",
                )
                for i in range(3)
            ]

            qctr = [0]

            def next_q(ndesc):
                qi = qctr[0] % nqueues
                qctr[0] += 1
                return qi

            def emit_layers(with_cc):
                for li, (w_sb, C, relu, hT_in, hT_out) in enumerate(layers):
                    gloc, gful = glocs[li], gfuls[li]

                    # stage A: g = dinv * (h @ W) for own nodes, bf16 table
                    for t in range(TPC):
                        psA = psA_pool.tile([128, C], dt.float32, tag="psA")
                        nc.tensor.matmul(
                            psA[:],
                            lhsT=hT_in[:, t * 128 : (t + 1) * 128],
                            rhs=w_sb[:, :C],
                            start=True,
                            stop=True,
                        )
                        # per-node (= per-partition) scale on the idle ACT
                        nc.scalar.mul(
                            g16_sb[:, t, :C], psA[:], dinv_sb[:, t : t + 1]
                        )
                    nc.sync.dma_start(
                        gloc[:].rearrange("(t p) c -> p t c", p=128),
                        g16_sb[:],
                    )

                    # stage B: replicate the g table
                    if with_cc:
                        nc.gpsimd.collective_compute(
                            "AllGather",
                            OP.bypass,
                            replica_groups=RG,
                            ins=[gloc[:]],
                            outs=[gful[:]],
                        )
                    g_lo = gful[0 : min(LO_CAP, TBL), :]
                    g_hi = gful[HI_OFF:TBL, :]

                    # stage C: batched gather + one-hot scatter per dest tile
                    woff = 0
                    for t0, gn in groups:
                        msg_lo = mlo_pool.tile(
                            [128, g_ * K_lo, CW], gdt, tag="mlo"
                        )
                        msg_hi = mhi_pool.tile(
                            [128, g_ * K_hi, CW], gdt, tag="mhi"
                        )
                        n_lo = gn * K_lo * 128
                        n_hi = gn * K_hi * 128
                        if 'nogather' not in abl:
                            # split the lo gather into two tile-halves so all
                            # calls are near-equal and round-robin queueing
                            # balances per-queue descriptor generation
                            h = gn // 2
                            for ta, tb in (((0, h) if h else None),
                                           (h, gn)):
                                if tb == ta:
                                    continue
                                n_ab = (tb - ta) * K_lo * 128
                                nc.gpsimd.dma_gather(
                                    msg_lo[:, ta * K_lo : tb * K_lo, :],
                                    g_lo[:],
                                    idx_sb[:, woff + ta * K_lo * 8 :
                                           woff + tb * K_lo * 8],
                                    n_ab,
                                    n_ab,
                                    CW,
                                    elem_step=CW,
                                    single_packet=('spkt' in abl),
                                    queue_num=next_q(n_ab),
                                )
                        woff += n_lo // 16
                        if 'nogather' not in abl:
                            nc.gpsimd.dma_gather(
                                msg_hi[:, 0 : gn * K_hi, :],
                                g_hi[:],
                                idx_sb[:, woff : woff + n_hi // 16],
                                n_hi,
                                n_hi,
                                CW,
                                elem_step=CW,
                                single_packet=('spkt' in abl),
                                queue_num=next_q(n_hi),
                            )
                        woff += n_hi // 16

                        for gi in range(gn):
                            t = t0 + gi
                            psS = psS_pool.tile([128, CW], dt.float32, tag="psS")
                            n_mm = 1 if 'nomm' in abl else K
                            for j in range(n_mm):
                                if 'nooh' in abl:
                                    mm_lhs = ident_sb if 'f32tbl' in abl else ident16_sb
                                else:
                                    oh = oh_pool.tile([128, 128], dt.bfloat16, tag="oh")
                                    nc.vector.tensor_scalar(
                                        oh[:],
                                        iota_sb[:],
                                        dloc_sb[:, t * K + j : t * K + j + 1],
                                        None,
                                        op0=OP.is_equal,
                                    )
                                    mm_lhs = oh
                                if 'nogather' in abl:
                                    mm_rhs = g16_sb[:, j, :]
                                elif j < K_lo:
                                    mm_rhs = msg_lo[:, gi * K_lo + j, :]
                                else:
                                    mm_rhs = msg_hi[:, gi * K_hi + (j - K_lo), :]
                                nc.tensor.matmul(
                                    psS[:],
                                    lhsT=mm_lhs[:],
                                    rhs=mm_rhs,
                                    start=(j == 0),
                                    stop=(j == n_mm - 1),
                                )
                            # epilogue: + self-loop, * dinv, (+bias), relu
                            acc = eps_pool.tile([128, C], dt.float32, tag="acc")
                            nc.vector.tensor_add(
                                acc[:], psS[:, :C], g16_sb[:, t, :C]
                            )
                            h_t = eps_pool.tile([128, C], dt.float32, tag="h_t")
                            if with_bias:
                                nc.vector.tensor_scalar_mul(
                                    acc[:], acc[:], dinv_sb[:, t : t + 1]
                                )
                                nc.vector.tensor_add(acc[:], acc[:], bias_sb[li][:])
                                if relu:
                                    nc.scalar.activation(h_t[:], acc[:], AF.Relu)
                                else:
                                    nc.scalar.copy(h_t[:], acc[:])
                            else:
                                if relu:
                                    nc.scalar.activation(
                                        h_t[:], acc[:], AF.Relu,
                                        scale=dinv_sb[:, t : t + 1],
                                    )
                                else:
                                    nc.scalar.mul(
                                        h_t[:], acc[:], dinv_sb[:, t : t + 1]
                                    )
                            if hT_out is not None:
                                psT = psT_pool.tile([128, 128], dt.float32, tag="psT")
                                nc.tensor.transpose(psT[:], h_t[:], ident_sb[:])
                                nc.vector.tensor_copy(
                                    hT_out[:, t * 128 : (t + 1) * 128], psT[:]
                                )
                            else:
                                nc.sync.dma_start(
                                    out_p[t * 128 : (t + 1) * 128, :], h_t[:]
                                )

            emit_layers(with_cc=True)
            if reps:
                with tc.For_i(0, reps, 1):
                    emit_layers(with_cc=False)

    nc.compile()
    return nc


_NC_CACHE: dict = {}


def _get_nc(K_lo, K_hi, with_bias):
    key = (K_lo, K_hi, with_bias)
    if key not in _NC_CACHE:
        _NC_CACHE[key] = _build_nc(K_lo, K_hi, with_bias)
    return _NC_CACHE[key]


# ---------------------------------------------------------------------------
# entry point
# ---------------------------------------------------------------------------
def _prepare(x, edge_index, W1, b1, W2, b2, W3, b3):
    x = np.asarray(x, dtype=np.float32)
    W1 = np.asarray(W1, dtype=np.float32)
    W2 = np.asarray(W2, dtype=np.float32)
    W3 = np.asarray(W3, dtype=np.float32)
    b1 = np.asarray(b1, dtype=np.float32)
    b2 = np.asarray(b2, dtype=np.float32)
    b3 = np.asarray(b3, dtype=np.float32)

    plan = _plan(np.asarray(edge_index))
    with_bias = bool(np.any(b1) or np.any(b2) or np.any(b3))
    nc = _get_nc(plan["K_lo"], plan["K_hi"], with_bias)

    iota = np.tile(np.arange(128, dtype=np.float32), (128, 1))
    ident = np.eye(128, dtype=np.float32)
    pos_local = plan["pos_local"]

    in_maps = []
    for k in range(M_CORES):
        pc = plan["per_core"][k]
        v0 = k * NPC
        xT = np.zeros((128, NPAD), dtype=np.float32)
        xT[:, pos_local[v0 : v0 + NPC]] = x[v0 : v0 + NPC].T
        im = dict(
            xT=xT, W1=W1, W2=W2, W3=W3,
            dinv=pc["dinv"], dloc=pc["dloc"], idx=pc["idx"],
            iota=iota, ident=ident,
        )
        if with_bias:
            im["b1r"] = np.tile(b1, (128, 1)).astype(np.float32)
            im["b2r"] = np.tile(b2, (128, 1)).astype(np.float32)
            im["b3r"] = np.tile(b3, (128, 1)).astype(np.float32)
        in_maps.append(im)

    def unpermute(results):
        out = np.empty((N_NODES, OUT_C), dtype=np.float32)
        for k in range(M_CORES):
            v0 = k * NPC
            r = results[k]["out"]
            out[v0 : v0 + NPC] = r[pos_local[v0 : v0 + NPC]]
        return out

    return nc, in_maps, unpermute


def kernel(x, edge_index, W1, b1, W2, b2, W3, b3):
    from concourse.bass_utils import run_bass_kernel_spmd

    nc, in_maps, unpermute = _prepare(x, edge_index, W1, b1, W2, b2, W3, b3)
    res = run_bass_kernel_spmd(nc, in_maps, list(range(M_CORES)))
    return unpermute(res.results)


# revision 21
# speedup vs baseline: 4.6817x; 1.6240x over previous
"""3-layer GCN node classifier on 8 Trainium2 NeuronCores.

Math (per layer, folding the symmetric normalization):
    deg[v]  = in-degree(v) (with self loop), dinv = rsqrt(deg)
    g       = dinv * (h @ W)                  (rows scaled)
    out[c]  = dinv[c] * ( sum_{e: col=c} g[row_e] + g[c] ) + b
    h_next  = relu(out)      (layers 1,2; layer 3 has no relu)

Distribution: nodes are range-sharded across 8 cores (graph parallel).
Each core computes g for its own nodes (dense matmul), the g-table is
all-gathered (bf16) to every core's HBM, each core then gathers the
rows for the edges whose *destination* it owns (SWDGE dma_gather,
batched over tile groups) and scatter-adds them into per-destination-
tile PSUM accumulators via one-hot bf16 matmuls on the TensorEngine.

Precision: h stays fp32 (stage-A matmuls fp32); only the replicated
message table g is rounded to bf16, and the scatter accumulation runs
in fp32 PSUM, so per-layer error is a single bf16 rounding (~0.4% el).

Host-side preprocessing only touches edge_index (graph structure):
CSR-style bucketing of edges by destination tile, degree computation,
a within-core node permutation that load-balances destination tiles,
and int16 gather-index packing (the 50k-row table is split into two
overlapping <=32768-row views because SWDGE gather indices are int16).
"""

import math
import os
import numpy as np

# ---------------------------------------------------------------------------
# problem constants (hardcoded per contract; kernel.py must be self-contained)
# ---------------------------------------------------------------------------
N_NODES = 50000
IN_C, HID_C, OUT_C = 128, 128, 64
M_CORES = 8
NPC = N_NODES // M_CORES            # 6250 nodes per core
TPC = (NPC + 127) // 128            # 49 destination tiles per core
NPAD = TPC * 128                    # 6272 padded nodes per core
TBL = M_CORES * NPAD                # 50176 rows in the all-gathered table
LO_CAP = 32768                      # int16 index reach
HI_OFF = max(0, TBL - 32768)        # 17408: hi view = table[HI_OFF:]
GRP = 8                             # dest tiles per batched SWDGE gather

F32 = "float32"


def _tile_groups():
    """[(tile_start, tile_count), ...] covering 0..TPC in GRP chunks."""
    return [(t0, min(GRP, TPC - t0)) for t0 in range(0, TPC, GRP)]


# ---------------------------------------------------------------------------
# host-side graph preprocessing (indices only)
# ---------------------------------------------------------------------------
def _plan(edge_index: np.ndarray):
    """Build per-core index/metadata arrays from edge_index [2, E]."""
    row = np.asarray(edge_index[0], dtype=np.int64)
    col = np.asarray(edge_index[1], dtype=np.int64)

    deg_in = np.bincount(col, minlength=N_NODES)          # edges only
    dinv = 1.0 / np.sqrt(deg_in + 1.0)                     # + self loop

    # within-core permutation: snake-fill tiles with degree-sorted nodes so
    # every destination tile carries a near-equal number of incoming edges.
    pos_local = np.empty(N_NODES, dtype=np.int64)
    for k in range(M_CORES):
        v0 = k * NPC
        d = deg_in[v0 : v0 + NPC]
        order = np.argsort(-d, kind="stable")              # heavy first
        # serpentine tile ids: 0..T-1, T-1..0, ...
        idx = np.arange(NPC)
        rnd, off = divmod(idx, TPC)
        tile_ids = np.where(rnd % 2 == 0, off, TPC - 1 - off)
        slot_in_tile = rnd
        pos = tile_ids * 128 + slot_in_tile
        pos_local[v0 + order] = pos

    g_pos = (np.arange(N_NODES) // NPC) * NPAD + pos_local  # table row per node

    # per-core per-tile edge buckets
    kd = col // NPC
    src_pos = g_pos[row]
    dst_slot = pos_local[col]
    dst_tile = dst_slot // 128
    dst_loc = dst_slot % 128
    is_lo = src_pos < LO_CAP

    # counts to size K_lo / K_hi uniformly across the SPMD program
    tile_key = kd * TPC + dst_tile
    n_lo = np.bincount(tile_key[is_lo], minlength=M_CORES * TPC)
    n_hi = np.bincount(tile_key[~is_lo], minlength=M_CORES * TPC)
    K_lo = max(1, int(math.ceil(n_lo.max() / 128)))
    K_hi = max(1, int(math.ceil(n_hi.max() / 128)))
    K = K_lo + K_hi
    NCH = TPC * K

    groups = _tile_groups()

    per_core = []
    for k in range(M_CORES):
        idx_lo = np.zeros((TPC, K_lo, 128), dtype=np.int16)  # pad -> row 0
        idx_hi = np.zeros((TPC, K_hi, 128), dtype=np.int16)
        dloc_a = np.full((TPC, K, 128), 200.0, dtype=np.float32)
        m = kd == k
        tl, lc, sp, lo = dst_tile[m], dst_loc[m], src_pos[m], is_lo[m]
        for t in range(TPC):
            mt = tl == t
            for stream, idx16, base in ((lo & mt, idx_lo, 0),
                                        ((~lo) & mt, idx_hi, K_lo)):
                sps = sp[stream]
                lcs = lc[stream]
                n = sps.size
                vals = sps if base == 0 else sps - HI_OFF
                idx16[t].reshape(-1)[:n] = vals.astype(np.int16)
                flat_d = dloc_a[t].reshape(-1)
                o = base * 128
                flat_d[o : o + n] = lcs.astype(np.float32)

        # batched-call slot order: per tile group, all lo chunks of its
        # tiles, then all hi chunks.  SWDGE wrapped-16 layout, replicated
        # across the 8 Q7 groups.
        parts = []
        for t0, gn in groups:
            parts.append(idx_lo[t0 : t0 + gn].reshape(-1))
            parts.append(idx_hi[t0 : t0 + gn].reshape(-1))
        flat = np.concatenate(parts)
        wrapped = flat.reshape(-1, 16).T                    # [16, slots/16]
        idx_sb = np.tile(wrapped, (8, 1)).copy()            # [128, slots/16]
        # per-chunk destination-local column, lane-major
        dloc_sb = dloc_a.transpose(2, 0, 1).reshape(128, NCH).copy()
        # per-slot dinv (0 on dummy slots)
        dv = np.zeros(NPAD, dtype=np.float32)
        v0 = k * NPC
        dv[pos_local[v0 : v0 + NPC]] = dinv[v0 : v0 + NPC]
        dinv_sb = dv.reshape(TPC, 128).T.copy()             # [128, TPC]
        per_core.append(dict(idx=idx_sb, dloc=dloc_sb, dinv=dinv_sb))

    return dict(
        K_lo=K_lo, K_hi=K_hi, NCH=NCH, per_core=per_core,
        pos_local=pos_local, dinv=dinv,
    )


# ---------------------------------------------------------------------------
# device program
# ---------------------------------------------------------------------------
def _build_nc(K_lo: int, K_hi: int, with_bias: bool, reps: int = 0, ablate: str = '',
              nqueues: int = 4, grp: int | None = None):
    """Build + compile the SPMD program.

    reps > 0 additionally emits a timing loop: the full pipeline runs once
    (correct, fills the gful tables), then a hardware For_i loop re-runs
    the whole body `reps` times with the collectives elided (collectives
    cannot sit inside control flow) so device time dominates wall clock.
    """
    abl = set(a for a in ablate.split(',') if a)
    import concourse.bacc as bacc
    import concourse.mybir as mybir
    from concourse import tile
    from concourse._compat import get_trn_type

    dt = mybir.dt
    K = K_lo + K_hi
    NCH = TPC * K
    NW = NCH * 8
    g_ = grp or GRP
    groups = [(t0, min(g_, TPC - t0)) for t0 in range(0, TPC, g_)]

    nc = bacc.Bacc(
        get_trn_type() or "TRN2",
        target_bir_lowering=False,
        debug=False,
        enable_asserts=False,
        num_devices=M_CORES,
        num_swdge_queues=nqueues,
    )

    # I/O
    xT_p = nc.dram_tensor("xT", [128, NPAD], dt.float32, kind="ExternalInput")
    W1_p = nc.dram_tensor("W1", [IN_C, HID_C], dt.float32, kind="ExternalInput")
    W2_p = nc.dram_tensor("W2", [HID_C, HID_C], dt.float32, kind="ExternalInput")
    W3_p = nc.dram_tensor("W3", [HID_C, OUT_C], dt.float32, kind="ExternalInput")
    dinv_p = nc.dram_tensor("dinv", [128, TPC], dt.float32, kind="ExternalInput")
    dloc_p = nc.dram_tensor("dloc", [128, NCH], dt.float32, kind="ExternalInput")
    idx_p = nc.dram_tensor("idx", [128, NW], dt.int16, kind="ExternalInput")
    iota_p = nc.dram_tensor("iota", [128, 128], dt.float32, kind="ExternalInput")
    ident_p = nc.dram_tensor("ident", [128, 128], dt.float32, kind="ExternalInput")
    if with_bias:
        b1_p = nc.dram_tensor("b1r", [128, HID_C], dt.float32, kind="ExternalInput")
        b2_p = nc.dram_tensor("b2r", [128, HID_C], dt.float32, kind="ExternalInput")
        b3_p = nc.dram_tensor("b3r", [128, OUT_C], dt.float32, kind="ExternalInput")
    out_p = nc.dram_tensor("out", [NPAD, OUT_C], dt.float32, kind="ExternalOutput")

    RG = [list(range(M_CORES))]
    AF = mybir.ActivationFunctionType
    OP = mybir.AluOpType

    with tile.TileContext(nc) as tc, tc.tile_pool(name="persist", bufs=1) as pp:
        # persistent SBUF tiles (one slot each)
        hT_a = pp.tile([128, NPAD], dt.float32, name="hT_a")
        hT_b = pp.tile([128, NPAD], dt.float32, name="hT_b")
        w1_sb = pp.tile([128, HID_C], dt.float32, name="w1_sb")
        w2_sb = pp.tile([128, HID_C], dt.float32, name="w2_sb")
        w3_sb = pp.tile([128, OUT_C], dt.float32, name="w3_sb")
        dinv_sb = pp.tile([128, TPC], dt.float32, name="dinv_sb")
        dloc_sb = pp.tile([128, NCH], dt.float32, name="dloc_sb")
        idx_sb = pp.tile([128, NW], dt.int16, name="idx_sb")
        iota_sb = pp.tile([128, 128], dt.bfloat16, name="iota_sb")
        ident_sb = pp.tile([128, 128], dt.float32, name="ident_sb")
        ident16_sb = pp.tile([128, 128], dt.bfloat16, name="ident16_sb")
        gdt = dt.float32 if 'f32tbl' in abl else dt.bfloat16
        g16_sb = pp.tile([128, TPC, HID_C], gdt, name="g16_sb")
        bias_sb = []

        nc.sync.dma_start(hT_a[:], xT_p[:])
        nc.sync.dma_start(w1_sb[:], W1_p[:])
        nc.sync.dma_start(w2_sb[:], W2_p[:])
        nc.sync.dma_start(w3_sb[:], W3_p[:])
        nc.sync.dma_start(dinv_sb[:], dinv_p[:])
        nc.sync.dma_start(dloc_sb[:], dloc_p[:])
        nc.sync.dma_start(idx_sb[:], idx_p[:])
        if with_bias:
            for p, cc in ((b1_p, HID_C), (b2_p, HID_C), (b3_p, OUT_C)):
                t = pp.tile([128, cc], dt.float32, name=f"bias{len(bias_sb)}_sb")
                nc.sync.dma_start(t[:], p[:])
                bias_sb.append(t)

        with tc.tile_pool(name="cvt", bufs=1) as cvt_pool:
            iota32 = cvt_pool.tile([128, 128], dt.float32, tag="iota32")
            nc.sync.dma_start(iota32[:], iota_p[:])
            nc.sync.dma_start(ident_sb[:], ident_p[:])
            nc.vector.tensor_copy(iota_sb[:], iota32[:])
            nc.vector.tensor_copy(ident16_sb[:], ident_sb[:])

        layers = [
            (w1_sb, HID_C, True, hT_a, hT_b),
            (w2_sb, HID_C, True, hT_b, hT_a),
            (w3_sb, OUT_C, False, hT_a, None),
        ]

        mbufs = 2 if 'bufs2' in abl else 3
        with (
            tc.tile_pool(name="mlo", bufs=mbufs) as mlo_pool,
            tc.tile_pool(name="mhi", bufs=mbufs) as mhi_pool,
            tc.tile_pool(name="oh", bufs=4) as oh_pool,
            tc.tile_pool(name="eps", bufs=3) as eps_pool,
            tc.tile_pool(name="psA", bufs=2, space="PSUM") as psA_pool,
            tc.tile_pool(name="psS", bufs=2, space="PSUM") as psS_pool,
            tc.tile_pool(name="psT", bufs=2, space="PSUM") as psT_pool,
            tc.tile_pool(name="dram", bufs=1, space="DRAM") as dram_pool,
        ):
            # All three tables are 128 wide (bf16 rows must be a multiple of
            # 256B for SWDGE).  Layer 3 only uses the first 64 columns; the
            # upper half carries stale layer-2 data that the epilogue never
            # reads.
            CW = HID_C
            glocs = [
                dram_pool.tile([NPAD, CW], gdt, name=f"gloc{i}")
                for i in range(3)
            ]
            gfuls = [
                dram_pool.tile(
                    [TBL, CW], gdt,
                    addr_space="Shared" if M_CORES > 4 else "Local",
                    name=f"gful{i}",
                )
                for i in range(3)
            ]

            qctr = [0]

            def next_q(ndesc):
                qi = qctr[0] % nqueues
                qctr[0] += 1
                return qi

            def emit_layers(with_cc):
                for li, (w_sb, C, relu, hT_in, hT_out) in enumerate(layers):
                    gloc, gful = glocs[li], gfuls[li]

                    # stage A: g = dinv * (h @ W) for own nodes, bf16 table
                    for t in range(TPC):
                        psA = psA_pool.tile([128, C], dt.float32, tag="psA")
                        nc.tensor.matmul(
                            psA[:],
                            lhsT=hT_in[:, t * 128 : (t + 1) * 128],
                            rhs=w_sb[:, :C],
                            start=True,
                            stop=True,
                        )
                        # per-node (= per-partition) scale on the idle ACT
                        nc.scalar.mul(
                            g16_sb[:, t, :C], psA[:], dinv_sb[:, t : t + 1]
                        )
                    nc.sync.dma_start(
                        gloc[:].rearrange("(t p) c -> p t c", p=128),
                        g16_sb[:],
                    )

                    # stage B: replicate the g table
                    if with_cc:
                        nc.gpsimd.collective_compute(
                            "AllGather",
                            OP.bypass,
                            replica_groups=RG,
                            ins=[gloc[:]],
                            outs=[gful[:]],
                        )
                    g_lo = gful[0 : min(LO_CAP, TBL), :]
                    g_hi = gful[HI_OFF:TBL, :]

                    # stage C: batched gather + one-hot scatter per dest tile
                    woff = 0
                    for t0, gn in groups:
                        msg_lo = mlo_pool.tile(
                            [128, g_ * K_lo, CW], gdt, tag="mlo"
                        )
                        msg_hi = mhi_pool.tile(
                            [128, g_ * K_hi, CW], gdt, tag="mhi"
                        )
                        n_lo = gn * K_lo * 128
                        n_hi = gn * K_hi * 128
                        if 'nogather' not in abl:
                            # split the lo gather into two tile-halves so all
                            # calls are near-equal and round-robin queueing
                            # balances per-queue descriptor generation
                            h = gn // 2
                            for ta, tb in ((0, h), (h, gn)):
                                if tb == ta:
                                    continue
                                n_ab = (tb - ta) * K_lo * 128
                                nc.gpsimd.dma_gather(
                                    msg_lo[:, ta * K_lo : tb * K_lo, :],
                                    g_lo[:],
                                    idx_sb[:, woff + ta * K_lo * 8 :
                                           woff + tb * K_lo * 8],
                                    n_ab,
                                    n_ab,
                                    CW,
                                    elem_step=CW,
                                    single_packet=('spkt' in abl),
                                    queue_num=next_q(n_ab),
                                )
                        woff += n_lo // 16
                        if 'nogather' not in abl:
                            nc.gpsimd.dma_gather(
                                msg_hi[:, 0 : gn * K_hi, :],
                                g_hi[:],
                                idx_sb[:, woff : woff + n_hi // 16],
                                n_hi,
                                n_hi,
                                CW,
                                elem_step=CW,
                                single_packet=('spkt' in abl),
                                queue_num=next_q(n_hi),
                            )
                        woff += n_hi // 16

                        for gi in range(gn):
                            t = t0 + gi
                            psS = psS_pool.tile([128, CW], dt.float32, tag="psS")
                            n_mm = 1 if 'nomm' in abl else K
                            for j in range(n_mm):
                                if 'nooh' in abl:
                                    mm_lhs = ident_sb if 'f32tbl' in abl else ident16_sb
                                else:
                                    oh = oh_pool.tile([128, 128], dt.bfloat16, tag="oh")
                                    nc.vector.tensor_scalar(
                                        oh[:],
                                        iota_sb[:],
                                        dloc_sb[:, t * K + j : t * K + j + 1],
                                        None,
                                        op0=OP.is_equal,
                                    )
                                    mm_lhs = oh
                                if 'nogather' in abl:
                                    mm_rhs = g16_sb[:, j, :]
                                elif j < K_lo:
                                    mm_rhs = msg_lo[:, gi * K_lo + j, :]
                                else:
                                    mm_rhs = msg_hi[:, gi * K_hi + (j - K_lo), :]
                                nc.tensor.matmul(
                                    psS[:],
                                    lhsT=mm_lhs[:],
                                    rhs=mm_rhs,
                                    start=(j == 0),
                                    stop=(j == n_mm - 1),
                                )
                            # epilogue: + self-loop, * dinv, (+bias), relu
                            acc = eps_pool.tile([128, C], dt.float32, tag="acc")
                            nc.vector.tensor_add(
                                acc[:], psS[:, :C], g16_sb[:, t, :C]
                            )
                            h_t = eps_pool.tile([128, C], dt.float32, tag="h_t")
                            if with_bias:
                                nc.vector.tensor_scalar_mul(
                                    acc[:], acc[:], dinv_sb[:, t : t + 1]
                                )
                                nc.vector.tensor_add(acc[:], acc[:], bias_sb[li][:])
                                if relu:
                                    nc.scalar.activation(h_t[:], acc[:], AF.Relu)
                                else:
                                    nc.scalar.copy(h_t[:], acc[:])
                            else:
                                if relu:
                                    nc.scalar.activation(
                                        h_t[:], acc[:], AF.Relu,
                                        scale=dinv_sb[:, t : t + 1],
                                    )
                                else:
                                    nc.scalar.mul(
                                        h_t[:], acc[:], dinv_sb[:, t : t + 1]
                                    )
                            if hT_out is not None:
                                psT = psT_pool.tile([128, 128], dt.float32, tag="psT")
                                nc.tensor.transpose(psT[:], h_t[:], ident_sb[:])
                                nc.vector.tensor_copy(
                                    hT_out[:, t * 128 : (t + 1) * 128], psT[:]
                                )
                            else:
                                nc.sync.dma_start(
                                    out_p[t * 128 : (t + 1) * 128, :], h_t[:]
                                )

            emit_layers(with_cc=True)
            if reps:
                with tc.For_i(0, reps, 1):
                    emit_layers(with_cc=False)

    nc.compile()
    return nc


_NC_CACHE: dict = {}


def _get_nc(K_lo, K_hi, with_bias):
    key = (K_lo, K_hi, with_bias)
    if key not in _NC_CACHE:
        _NC_CACHE[key] = _build_nc(K_lo, K_hi, with_bias)
    return _NC_CACHE[key]


# ---------------------------------------------------------------------------
# entry point
# ---------------------------------------------------------------------------
def _prepare(x, edge_index, W1, b1, W2, b2, W3, b3):
    x = np.asarray(x, dtype=np.float32)
    W1 = np.asarray(W1, dtype=np.float32)
    W2 = np.asarray(W2, dtype=np.float32)
    W3 = np.asarray(W3, dtype=np.float32)
    b1 = np.asarray(b1, dtype=np.float32)
    b2 = np.asarray(b2, dtype=np.float32)
    b3 = np.asarray(b3, dtype=np.float32)

    plan = _plan(np.asarray(edge_index))
    with_bias = bool(np.any(b1) or np.any(b2) or np.any(b3))
    nc = _get_nc(plan["K_lo"], plan["K_hi"], with_bias)

    iota = np.tile(np.arange(128, dtype=np.float32), (128, 1))
    ident = np.eye(128, dtype=np.float32)
    pos_local = plan["pos_local"]

    in_maps = []
    for k in range(M_CORES):
        pc = plan["per_core"][k]
        v0 = k * NPC
        xT = np.zeros((128, NPAD), dtype=np.float32)
        xT[:, pos_local[v0 : v0 + NPC]] = x[v0 : v0 + NPC].T
        im = dict(
            xT=xT, W1=W1, W2=W2, W3=W3,
            dinv=pc["dinv"], dloc=pc["dloc"], idx=pc["idx"],
            iota=iota, ident=ident,
        )
        if with_bias:
            im["b1r"] = np.tile(b1, (128, 1)).astype(np.float32)
            im["b2r"] = np.tile(b2, (128, 1)).astype(np.float32)
            im["b3r"] = np.tile(b3, (128, 1)).astype(np.float32)
        in_maps.append(im)

    def unpermute(results):
        out = np.empty((N_NODES, OUT_C), dtype=np.float32)
        for k in range(M_CORES):
            v0 = k * NPC
            r = results[k]["out"]
            out[v0 : v0 + NPC] = r[pos_local[v0 : v0 + NPC]]
        return out

    return nc, in_maps, unpermute


def kernel(x, edge_index, W1, b1, W2, b2, W3, b3):
    from concourse.bass_utils import run_bass_kernel_spmd

    nc, in_maps, unpermute = _prepare(x, edge_index, W1, b1, W2, b2, W3, b3)
    res = run_bass_kernel_spmd(nc, in_maps, list(range(M_CORES)))
    return unpermute(res.results)
